# revision 1
# baseline (speedup 1.0000x reference)
"""MultiLobeSGGX.sample() Trainium2 Bass kernel v2 — single-pass fp16 tiles.

Per 128xF tile: luminance routing (custom LUM2 + STT), sample remap via
fused seed+1NR divide-clip custom DVE ops, merged-basis SGGX VNDF specular +
uniform-hemisphere diffuse in fp16 with CP selects, f32 only where range
demands (mask compare, n2/r2, den/rden). Engine split: DVE fp16 TT/TS +
customs + CP, Pool f32 adds/compares + OUT writes, ACT unary funcs + dtype
conversions.
"""
import sys
sys.path.insert(0, '/opt/trn_rl_repo')
import numpy as np
import concourse.bass as bass
import concourse.bacc as bacc
import concourse.mybir as mybir
import concourse.tile as tile

dt = mybir.dt.float32
hp = mybir.dt.float16
A = mybir.ActivationFunctionType
OP = mybir.AluOpType
PI = float(np.pi)
EPS = 1e-6

# ---------------- custom DVE ops (registered into dve_ops at import) -------
import concourse.dve_ops as dops
from concourse.dve_spec import (Spec, Src0, Src1, C0, C1, C2, Bin, AluOp,
                                minn, maxx, Zero, One, lower, _has_src1)
from concourse.dve_uop import DveOpSpec
from concourse.dve_table_gen import dve_ver_for

_RC0 = -0.23549792   # recip seed Chebyshev consts (see dve_ops.py)
_RC1 = 2.0017324


def _recip1(x):
    """seed + 1 Newton-Raphson pass on node x (5 pipeline stages)."""
    nx = Bin(AluOp.BITWISE_NOT, x, x)
    y0 = nx * C0
    return y0 * (C1 - x * y0)


def _np_recip1(x, c0=_RC0, c1=_RC1):
    x = np.asarray(x, np.float32)
    nx = (~x.view(np.int32)).view(np.float32)
    y0 = nx * np.float32(c0)
    return y0 * (np.float32(c1) - x * y0)


def _dve_max(a, b):
    r = np.maximum(a, b)
    r = np.where(np.isnan(a), b, r)
    return np.where(np.isnan(b), a, r)


def _dve_min(a, b):
    r = np.minimum(a, b)
    r = np.where(np.isnan(a), b, r)
    return np.where(np.isnan(b), a, r)


def _ref_divclip(in0, in1, s0, s1, imm2):
    y = _np_recip1(in1, s0, s1)
    p = in0.astype(np.float32) * y
    return _dve_min(_dve_max(p, np.float32(0.0)), np.float32(1.0))


def _ref_divbclip(in0, in1, s0, s1, imm2):
    y = _np_recip1(np.asarray(in1, np.float32) + np.float32(imm2), s0, s1)
    p = in0.astype(np.float32) * y
    return _dve_min(p, np.float32(1.0))


def _ref_lum2(in0, in1, s0, s1, imm2):
    return in0.astype(np.float32) * np.float32(s0) + in1.astype(np.float32) * np.float32(s1)


def _register(name, spec):
    if name in dops._SUB_OPCODE_FOR_NAME:
        return next(o for o in dops.OPS if o.name == name)
    row = dops._CUSTOM_DVE_ROW_BASE + len(dops.OPS)
    assert row < 0x20
    ver = dve_ver_for("TRN2")
    tmp = DveOpSpec(name=name, opcode=row, uops=lower(spec, ver=ver),
                    rd1_en=_has_src1(spec))
    op = dops.DveOp(name, spec, False, {ver: tmp.sha(ver)})
    dops.OPS.append(op)
    dops._SUB_OPCODE_FOR_NAME[name] = row
    dops.CUSTOM_DVE_SPECS[name] = spec
    return op


# out = clip01(in0 * recip1(in1)); s0/s1 = recip seed consts
DIVCLIP = _register("ANT_DIVCLIP01", Spec(
    body=minn(maxx(Src0 * _recip1(Src1), Zero), One),
    reference=_ref_divclip))
# out = min(in0 * recip1(in1 + imm2), 1)  (no lower clip)
DIVBCLIP = _register("ANT_DIVBCLIP1", Spec(
    body=minn(Src0 * _recip1(Src1 + C2), One),
    reference=_ref_divbclip))
# out = in0*s0 + in1*s1
LUM2 = _register("ANT_LUM2", Spec(
    body=Src0 * C0 + Src1 * C1,
    reference=_ref_lum2))




def _ref_sinh(in0, in1, s0, s1, imm2):
    z = np.asarray(in0, np.float32)
    y = z * (np.float32(1.0) - np.abs(z))
    return y * (np.float32(s1) + np.float32(s0) * np.abs(y))


# out = y*(C1 + C0*|y|), y = z(1-|z|)  — with C0=-3.6, C1=-3.1 gives -sin(pi z)
_az = Bin(AluOp.ABSOLUTE_VALUE, Src0, Src0)
_ys = Src0 * (One - _az)
_ay = Bin(AluOp.ABSOLUTE_VALUE, _ys, _ys)
SINH = _register("ANT_SINHALF", Spec(
    body=_ys * (C1 + _ay * C0),
    reference=_ref_sinh))




def _ref_div2s(in0, in1, s0, s1, imm2):
    y = _np_recip1(in1, s0, s1)
    return (np.asarray(in0, np.float32) * y) * np.float32(imm2)


DIV2S = _register("ANT_DIV2S", Spec(
    body=(Src0 * _recip1(Src1)) * C2,
    reference=_ref_div2s))


def _ref_sqsum2(in0, in1, s0, s1, imm2):
    a = np.asarray(in0, np.float32); b = np.asarray(in1, np.float32)
    return a * a + b * b


SQSUM2 = _register("ANT_SQSUM2", Spec(
    body=Src0 * Src0 + Src1 * Src1,
    reference=_ref_sqsum2))


def _ref_tq1(in0, in1, s0, s1, imm2):
    c2 = np.asarray(in1, np.float32)
    n2 = np.asarray(in0, np.float32) + c2 * c2
    return c2 * _np_recip1(n2, s0, s1)


TQ1 = _register("ANT_TQ1", Spec(
    body=Src1 * _recip1(Src0 + Src1 * Src1),
    reference=_ref_tq1))


def _ref_m2t(in0, in1, s0, s1, imm2):
    r = np.float32(1.0) - np.asarray(in0, np.float32) * np.asarray(in1, np.float32)
    return _dve_max(r, np.float32(0.0))


M2T = _register("ANT_M2T", Spec(
    body=maxx(One - Src0 * Src1, Zero),
    reference=_ref_m2t))


def _ref_sii(in0, in1, s0, s1, imm2):
    a2 = np.asarray(in1, np.float32)
    return a2 + (np.float32(1.0) - a2) * np.square(np.asarray(in0, np.float32))


SII = _register("ANT_SII", Spec(
    body=Src1 + (One - Src1) * (Src0 * Src0),
    reference=_ref_sii))


def _ref_rden(in0, in1, s0, s1, imm2):
    x = np.asarray(in0, np.float32) + np.asarray(in1, np.float32)
    return _np_recip1(x, s0, s1)


RDEN = _register("ANT_RDEN", Spec(
    body=_recip1(Src0 + Src1),
    reference=_ref_rden))


class Scratch:
    """Fixed slot pool with FIFO reuse (spreads WAR deps across slots)."""

    def __init__(self, pool, F, n, dtype, pfx):
        self.slots = [pool.tile([128, F], dtype, tag=f"{pfx}{i}", name=f"{pfx}{i}")
                      for i in range(n)]
        self.free = list(range(n))
        self.used = {}

    def get(self, nm):
        i = self.free.pop(0)
        self.used[nm] = i
        return self.slots[i][:]

    def rel(self, *nms):
        for nm in nms:
            self.free.append(self.used.pop(nm))

    def __getitem__(self, nm):
        return self.slots[self.used[nm]][:]


# default engine per op site: "V"=DVE, "P"=Pool, "A"=ACT handled separately
DEFAULT_ENG = dict(
    tote="P", tp="P", mask="V", num="P", carg="V", den="P", tsq="P",
    wadj="V", n2a="P", n2="P", outx="P", outy="P", outz="P",
    conv="A",
)


def build_module(R, F=512, num_devices=8, io_bufs=2, sc_bufs=1,
                 nsc16=30, nsc32=8, eng=None, custom_fp16_out=False,
                 use_tq2_stt=True, repeat=1):
    E = dict(DEFAULT_ENG)
    if eng:
        E.update(eng)
    assert R % (128 * F) == 0
    T = R // (128 * F)

    nc = bacc.Bacc("TRN2", target_bir_lowering=False, debug=False,
                   num_devices=num_devices)

    def register_const(val):
        t = nc.alloc_sbuf_tensor(f"uconst-{val}", [128, 1], dt)
        nc.gpsimd.memset(t.ap(), val)
        nc.const_aps.aps[(dt, float(val))] = t.ap()
    for v in (PI, PI / 2, 1.0, EPS):
        register_const(v)
    nc.all_engine_barrier()

    wi = nc.dram_tensor("wi", [R, 3], dt, kind="ExternalInput")
    nn = nc.dram_tensor("n", [R, 3], dt, kind="ExternalInput")
    al = nc.dram_tensor("albedo", [R, 3], dt, kind="ExternalInput")
    me = nc.dram_tensor("metallic", [R, 3], dt, kind="ExternalInput")
    ax = nc.dram_tensor("alpha_x", [R, 1], dt, kind="ExternalInput")
    sa = nc.dram_tensor("sample", [R, 2], dt, kind="ExternalInput")
    wo = nc.dram_tensor("wo", [R, 3], dt, kind="ExternalOutput")

    wi_r = wi[:, :].rearrange("(t p f) c -> t p (f c)", p=128, f=F)
    nn_r = nn[:, :].rearrange("(t p f) c -> t p (f c)", p=128, f=F)
    al_r = al[:, :].rearrange("(t p f) c -> t p (f c)", p=128, f=F)
    me_r = me[:, :].rearrange("(t p f) c -> t p (f c)", p=128, f=F)
    ax_r = ax[:, :].rearrange("(t p f) 1 -> t p f", p=128, f=F)
    sa_r = sa[:, :].rearrange("(t p f) c -> t p (f c)", p=128, f=F)
    wo_r = wo[:, :].rearrange("(t p f) c -> t p (f c)", p=128, f=F)

    ve = nc.vector
    ge = nc.gpsimd
    ae = nc.scalar

    def engine(site):
        return ve if E.get(site, "V") == "V" else ge

    with tile.TileContext(nc) as tc:
        with (
            tc.tile_pool(name="io", bufs=io_bufs) as pio,
            tc.tile_pool(name="sc16", bufs=sc_bufs) as p16,
            tc.tile_pool(name="sc32", bufs=sc_bufs) as p32,
        ):
            S16 = Scratch(p16, F, nsc16, hp, "h")
            S32 = Scratch(p32, F, nsc32, dt, "f")
            wide = E.get("wide", 0)

            def emit_p1(tg):
                AL = pio.tile([128, 3 * F], dt, tag="AL", name="AL")
                ME = pio.tile([128, 3 * F], dt, tag="ME", name="ME")
                SA = pio.tile([128, 2 * F], dt, tag="SA", name="SA")
                WI = pio.tile([128, 3 * F], dt, tag="WI", name="WI")
                NN = pio.tile([128, 3 * F], dt, tag="NN", name="NN")
                ALP = pio.tile([128, F], dt, tag="ALP", name="ALP")
                dq = {"S": nc.sync, "A": nc.scalar, "G": nc.gpsimd,
                      "V": nc.vector}[E.get("dmaq", "S")]
                dq2 = {"S": nc.sync, "A": nc.scalar, "G": nc.gpsimd,
                       "V": nc.vector}[E.get("dmaq2", "A")]
                dq.dma_start(AL[:], al_r[tg])
                dq.dma_start(ME[:], me_r[tg])
                dq.dma_start(SA[:], sa_r[tg])
                dq2.dma_start(WI[:], wi_r[tg])
                dq2.dma_start(NN[:], nn_r[tg])
                dq.dma_start(ALP[:], ax_r[tg])
                ALv = AL[:].rearrange("p (f c) -> p f c", c=3)
                MEv = ME[:].rearrange("p (f c) -> p f c", c=3)
                SAv = SA[:].rearrange("p (f c) -> p f c", c=2)
                s0 = SAv[:, :, 0]
                s1 = SAv[:, :, 1]

                # ---------- phase 1: routing ----------
                wd = S32.get("wd"); ws = S32.get("ws")
                if E.get("lum", "V") == "P":
                    for out, V3 in ((wd, ALv), (ws, MEv)):
                        t1_ = S32.get("lumt1"); t2_ = S32.get("lumt2")
                        ge.tensor_scalar(t1_, V3[:, :, 0], 0.2126, None, op0=OP.mult)
                        ge.tensor_scalar(t2_, V3[:, :, 1], 0.7152, None, op0=OP.mult)
                        ge.tensor_tensor(t1_, t1_, t2_, OP.add)
                        ge.tensor_scalar(t2_, V3[:, :, 2], 0.0722, None, op0=OP.mult)
                        ge.tensor_tensor(out, t1_, t2_, OP.add)
                        S32.rel("lumt1", "lumt2")
                else:
                    ve._custom_dve(LUM2, out=wd, in0=ALv[:, :, 0], in1=ALv[:, :, 1],
                                   s0=0.2126, s1=0.7152)
                    ve.scalar_tensor_tensor(wd, ALv[:, :, 2], 0.0722, wd, OP.mult, OP.add)
                    ve._custom_dve(LUM2, out=ws, in0=MEv[:, :, 0], in1=MEv[:, :, 1],
                                   s0=0.2126, s1=0.7152)
                    ve.scalar_tensor_tensor(ws, MEv[:, :, 2], 0.0722, ws, OP.mult, OP.add)
                tote = S32.get("tote")
                engine("tote").tensor_tensor(tote, wd, ws, OP.add)
                tp = S32.get("tp")
                engine("tp").tensor_tensor(tp, s0, tote, OP.mult)
                S32.rel("tote")
                MQ = pio.tile([128, F], mybir.dt.uint8, tag="MQ", name="MQ")
                mask = MQ[:]
                if E.get("masksplit", 0):
                    mf = S32.get("mf")
                    ge.tensor_tensor(mf, ws, tp, OP.is_gt)
                    if E.get("masksplit") == "A":
                        ae.activation(mask, mf, A.Copy)
                    else:
                        ve.tensor_scalar(mask, mf, 0.0, None, op0=OP.add)
                    S32.rel("mf")
                else:
                    engine("mask").tensor_tensor(mask, ws, tp, OP.is_gt)

                U0S = pio.tile([128, F], hp if custom_fp16_out else dt,
                               tag="U0S", name="U0S")
                u0s = U0S[:]
                ve._custom_dve(DIVCLIP, out=u0s, in0=tp, in1=ws, s0=_RC0, s1=_RC1)
                num = S32.get("num")
                engine("num").tensor_tensor(num, tp, ws, OP.subtract)
                S32.rel("tp")
                U0D = pio.tile([128, F], hp if custom_fp16_out else dt,
                               tag="U0D", name="U0D")
                u0d = U0D[:]
                ve._custom_dve(DIVBCLIP, out=u0d, in0=num, in1=wd,
                               s0=_RC0, s1=_RC1, imm2=EPS)
                S32.rel("wd", "ws", "num")

                SB = pio.tile([128, F], hp, tag="SB", name="SB")
                sinb = SB[:]
                CB = pio.tile([128, F], hp, tag="CB", name="CB")
                cosb = CB[:]
                if E.get("sin", "A") == "A":
                    ae.activation(sinb, s1, A.Sin, scale=-2.0 * PI, bias=PI)
                    wadj = S32.get("wadj")
                    engine("wadj").tensor_scalar(wadj, s1, 0.75, None, op0=OP.is_gt)
                    carg = S32.get("carg")
                    engine("carg").tensor_tensor(carg, s1, wadj, OP.subtract)
                    S32.rel("wadj")
                    ae.activation(cosb, carg, A.Sin, scale=-2.0 * PI, bias=PI / 2)
                    S32.rel("carg")
                else:
                    # sin(2*pi*u) = -sin(pi*(2u-1)) = SINH(2u-1)
                    zs = S16.get("zs")
                    ve.tensor_scalar(zs, s1, 2.0, -1.0, op0=OP.mult, op1=OP.add)
                    ve._custom_dve(SINH, out=sinb, in0=zs, s0=-3.6, s1=-3.1)
                    S16.rel("zs")
                    # cos(2*pi*u) = -sin(pi*zc), zc = 2u-0.5-2*[u>=0.25]... use
                    # zc = 2u+0.5 - 2*[u >= 0.25] in [-0.5, 1)-ish:
                    # cos(2piu) = sin(2pi(u+0.25)) ; v = u+0.25 wrapped to [0,1)
                    # = -sin(pi*(2v-1)); v = u+0.25 - [u >= 0.75]
                    hq = S16.get("hq")
                    ve.tensor_scalar(hq, s1, 0.75, 2.0, op0=OP.is_ge, op1=OP.mult)
                    zc = S16.get("zc")
                    ve.tensor_scalar(zc, s1, 2.0, -0.5, op0=OP.mult, op1=OP.add)
                    zc2 = S16.get("zc2")
                    ve.tensor_tensor(zc2, zc, hq, OP.subtract)
                    S16.rel("hq", "zc")
                    ve._custom_dve(SINH, out=cosb, in0=zc2, s0=-3.6, s1=-3.1)
                    S16.rel("zc2")
                return dict(mask=mask, u0s=u0s, u0d=u0d, sinb=sinb, cosb=cosb,
                            WI=WI, NN=NN, ALP=ALP)

            def emit_p2a(tg, ph1):
                sfx = f"@{tg % 2}"
                mask = ph1["mask"]; u0s = ph1["u0s"]; u0d = ph1["u0d"]
                sinb = ph1["sinb"]; cosb = ph1["cosb"]
                WI = ph1["WI"]; NN = ph1["NN"]; ALP = ph1["ALP"]
                WIv = WI[:].rearrange("p (f c) -> p f c", c=3)
                NNv = NN[:].rearrange("p (f c) -> p f c", c=3)
                OUT = pio.tile([128, 3 * F], dt, tag="OUT", name="OUT")
                OUTv = OUT[:].rearrange("p (f c) -> p f c", c=3)

                # ---------- phase 2: geometry ----------
                n16 = [S16.get(f"n16{c}") for c in range(3)]
                w16 = [S16.get(f"w16{c}") for c in range(3)]
                b = [S16.get(f"b{c}" + sfx) for c in range(3)]
                for c in range(3):
                    if E.get("conv", "A") == "A":
                        ae.activation(n16[c], NNv[:, :, c], A.Copy)
                        ae.activation(w16[c], WIv[:, :, c], A.Copy)
                    else:
                        cv = ve if E.get("conv") == "V" else ge
                        cv.tensor_scalar(n16[c], NNv[:, :, c], 0.0, None, op0=OP.add)
                        cv.tensor_scalar(w16[c], WIv[:, :, c], 0.0, None, op0=OP.add)
                    if E.get("bcopy", "V") == "A":
                        ae.activation(b[c], n16[c], A.Copy)
                    else:
                        ve.tensor_scalar(b[c], n16[c], 0.0, None, op0=OP.add)
                    ve.copy_predicated(b[c], mask, w16[c])
                bx, by, bz = b
                nx, ny, nz = n16
                for c in range(3):
                    S16.rel(f"w16{c}")

                sgn = S16.get("sgn" + sfx)
                if E.get("sgnact", 0):
                    ae.activation(sgn, bz, A.Sign, bias=EPS)
                else:
                    h2 = S16.get("h2")
                    ve.tensor_scalar(h2, bz, 0.0, 2.0, op0=OP.is_ge, op1=OP.mult)
                    ve.tensor_scalar(sgn, h2, -1.0, None, op0=OP.add)
                    S16.rel("h2")
                if E.get("rden", "C") == "C":
                    rd16 = S16.get("rd16" + sfx)
                    ve._custom_dve(RDEN, out=rd16, in0=sgn, in1=bz,
                                   s0=_RC0, s1=_RC1)
                else:
                    den = S32.get("den")
                    engine("den").tensor_tensor(den, sgn, bz, OP.add)
                    rden = S32.get("rden")
                    ve.reciprocal_approx_fast(rden, den)
                    S32.rel("den")
                    rd16 = S16.get("rd16" + sfx)
                    ve.tensor_scalar(rd16, rden, 0.0, None, op0=OP.add)
                    S32.rel("rden")

                u = S16.get("u" + sfx)
                ve.tensor_tensor(u, sgn, bx, OP.mult)
                dta = S16.get("dta"); dtb = S16.get("dtb")
                engine("dta").tensor_tensor(dta, bx, nx, OP.mult)
                engine("dtb").tensor_tensor(dtb, by, ny, OP.mult)
                d2 = S16.get("d2")
                ve.tensor_tensor(d2, dta, dtb, OP.add)
                S16.rel("dta", "dtb")
                qd = S16.get("qd")
                ve.tensor_tensor(qd, d2, rd16, OP.mult)
                Q = S16.get("Q")
                engine("qQ").tensor_tensor(Q, qd, nz, OP.add)
                S16.rel("qd")
                tk = S16.get("tk")
                ve.tensor_tensor(tk, u, Q, OP.mult)
                kn = S16.get("kn")
                ve.tensor_tensor(kn, nx, tk, OP.subtract)
                S16.rel("tk")
                tj = S16.get("tj"); tj2 = S16.get("tj2")
                engine("tj").tensor_tensor(tj, sgn, ny, OP.mult)
                ve.tensor_tensor(tj2, by, Q, OP.mult)
                jn = S16.get("jn")
                ve.tensor_tensor(jn, tj, tj2, OP.subtract)
                S16.rel("tj", "tj2", "Q")
                ti = S16.get("ti")
                engine("ti").tensor_tensor(ti, bz, nz, OP.mult)
                inn = S16.get("inn")
                engine("inn").tensor_tensor(inn, d2, ti, OP.add)
                S16.rel("ti", "d2")
                for c in range(3):
                    S16.rel(f"n16{c}")

                a2 = S16.get("a2")
                ae.activation(a2, ALP[:], A.Square)
                beta = S16.get("beta")
                ve.tensor_scalar(beta, a2, -1.0, 1.0, op0=OP.mult, op1=OP.add)
                bk = S16.get("bk")
                engine("bk").tensor_tensor(bk, beta, kn, OP.mult)
                if E.get("m2tc", 0):
                    m2t = S16.get("m2t")
                    ve._custom_dve(M2T, out=m2t, in0=bk, in1=kn)
                    S16.rel("kn")
                    mS = S16.get("mS")
                    ae.activation(mS, m2t, A.Sqrt)
                else:
                    bkk = S16.get("bkk")
                    ve.tensor_tensor(bkk, bk, kn, OP.mult)
                    S16.rel("kn")
                if E.get("m2tc", 0):
                    pass
                elif E.get("m2trelu", 0):
                    m2t = S16.get("m2t")
                    ae.activation(m2t, bkk, A.Relu, scale=-1.0, bias=1.0)
                    S16.rel("bkk")
                    mS = S16.get("mS")
                    ae.activation(mS, m2t, A.Sqrt)
                else:
                    bkkc = S16.get("bkkc")
                    ve.tensor_scalar(bkkc, bkk, 1.0, None, op0=OP.min)
                    S16.rel("bkk")
                    mS = S16.get("mS")
                    ae.activation(mS, bkkc, A.Sqrt, scale=-1.0, bias=1.0)
                    m2t = S16.get("m2t")
                    ve.tensor_scalar(m2t, bkkc, -1.0, 1.0, op0=OP.mult, op1=OP.add)
                    S16.rel("bkkc")
                if E.get("bicut", 0):
                    bi = None
                    inn2 = S16.get("inn2")
                    ae.activation(inn2, inn, A.Square)
                    bii = S16.get("bii")
                    ve.tensor_tensor(bii, beta, inn2, OP.mult)
                    S16.rel("inn2")
                else:
                    bi = S16.get("bi")
                    engine("bi").tensor_tensor(bi, beta, inn, OP.mult)
                    S16.rel("beta")
                    bii = S16.get("bii")
                    ve.tensor_tensor(bii, bi, inn, OP.mult)
                Sii = S16.get("Sii")
                if E.get("siic", 0):
                    ve._custom_dve(SII, out=Sii, in0=inn, in1=a2)
                    S16.rel("bii")
                else:
                    ve.tensor_tensor(Sii, bii, a2, OP.add)
                    S16.rel("bii")
                sqS = S16.get("sqS")
                ae.activation(sqS, Sii, A.Sqrt)

                tsq = S16.get("tsq")
                engine("tsq").tensor_tensor(tsq, u0s, a2, OP.mult)
                S16.rel("a2")
                squa = S16.get("squa")
                ae.activation(squa, tsq, A.Sqrt)
                S16.rel("tsq")
                ww = S16.get("ww")
                ae.activation(ww, u0s, A.Sqrt, scale=-1.0, bias=1.0)
                z2 = S16.get("z2")
                ae.activation(z2, u0d, A.Square)
                if E.get("zrelu", "V") == "A":
                    z2c = S16.get("z2c")
                    ae.activation(z2c, z2, A.Relu, scale=-1.0, bias=1.0)
                    S16.rel("z2")
                    rd_ = S16.get("rd_")
                    ae.activation(rd_, z2c, A.Sqrt)
                    S16.rel("z2c")
                else:
                    z2c = S16.get("z2c")
                    ve.tensor_scalar(z2c, z2, 1.0, None, op0=OP.min)
                    S16.rel("z2")
                    rd_ = S16.get("rd_")
                    ae.activation(rd_, z2c, A.Sqrt, scale=-1.0, bias=1.0)
                    S16.rel("z2c")

                vv = S16.get("vv")
                ve.tensor_tensor(vv, squa, sinb, OP.mult)
                sqc = S16.get("sqc")
                ve.tensor_tensor(sqc, squa, sqS, OP.mult)
                S16.rel("squa", "sqS")
                usq = S16.get("usq")
                engine("usq").tensor_tensor(usq, sqc, cosb, OP.mult)
                S16.rel("sqc")
                if wide:
                    DIF = p16.tile([128, 3 * F], hp, tag=f"DIF{tg % 2}",
                                   name=f"DIF{tg % 2}")
                    e0d = DIF[:, 0:F]; e1d = DIF[:, F:2 * F]
                else:
                    DIF = None
                    e0d = S16.get("e0d" + sfx); e1d = S16.get("e1d" + sfx)
                engine("e0d").tensor_tensor(e0d, rd_, cosb, OP.mult)
                engine("e1d").tensor_tensor(e1d, rd_, sinb, OP.mult)
                S16.rel("rd_")
                wm_ = S16.get("wm_")
                ve.tensor_tensor(wm_, ww, mS, OP.mult)
                S16.rel("ww", "mS")

                p1_ = S16.get("p1_"); p2_ = S16.get("p2_")
                ve.tensor_tensor(p1_, vv, jn, OP.mult)
                engine("p2_").tensor_tensor(p2_, wm_, inn, OP.mult)
                _bicut = E.get("bicut", 0)
                P_ = S16.get("P_")
                ve.tensor_tensor(P_, p1_, p2_, OP.add)
                if _bicut:
                    S16.rel("p1_", "inn")
                else:
                    S16.rel("p1_", "p2_", "inn")
                tc0 = S16.get("tc0")
                ve.tensor_tensor(tc0, bk, P_, OP.mult)
                S16.rel("bk", "P_")
                if wide:
                    CBLK = p16.tile([128, 3 * F], hp, tag=f"CBLK{tg % 2}",
                                    name=f"CBLK{tg % 2}")
                    c0 = CBLK[:, 0:F]
                else:
                    CBLK = None
                    c0 = S16.get("c0" + sfx)
                ve.tensor_tensor(c0, tc0, usq, OP.add)
                S16.rel("tc0", "usq")
                if not _bicut:
                    W2 = S16.get("W2")
                    ve.tensor_tensor(W2, wm_, jn, OP.mult)
                    S16.rel("jn")
                t3 = S16.get("t3"); t4 = S16.get("t4")
                ve.tensor_tensor(t3, vv, m2t, OP.mult)
                S16.rel("vv", "m2t")
                if _bicut:
                    tpj = S16.get("tpj")
                    ve.tensor_tensor(tpj, p2_, jn, OP.mult)
                    S16.rel("jn", "p2_")
                    ve.tensor_tensor(t4, beta, tpj, OP.mult)
                    S16.rel("tpj", "beta")
                else:
                    ve.tensor_tensor(t4, bi, W2, OP.mult)
                    S16.rel("bi", "W2")
                c1 = CBLK[:, F:2 * F] if wide else S16.get("c1" + sfx)
                ve.tensor_tensor(c1, t3, t4, OP.add)
                S16.rel("t3", "t4")
                c2 = CBLK[:, 2 * F:3 * F] if wide else S16.get("c2" + sfx)
                ve.tensor_tensor(c2, wm_, Sii, OP.mult)
                S16.rel("wm_", "Sii")

                if E.get("tqfuse", 0):
                    n2 = S32.get("n2" + sfx)
                    ve._custom_dve(SQSUM2, out=n2, in0=c0, in1=c1)
                elif wide:
                    SQW = p32.tile([128, 3 * F], dt, tag=f"SQW{tg % 2}",
                                   name=f"SQW{tg % 2}")
                    ae.activation(SQW[:], CBLK[:], A.Square)
                    n2a = S32.get("n2a")
                    engine("n2a").tensor_tensor(n2a, SQW[:, 0:F], SQW[:, F:2 * F], OP.add)
                    n2 = S32.get("n2" + sfx)
                    engine("n2").tensor_tensor(n2, n2a, SQW[:, 2 * F:3 * F], OP.add)
                    S32.rel("n2a")
                else:
                    c0s = S32.get("c0s"); c1s = S32.get("c1s"); c2s = S32.get("c2s")
                    ae.activation(c0s, c0, A.Square)
                    ae.activation(c1s, c1, A.Square)
                    ae.activation(c2s, c2, A.Square)
                    n2a = S32.get("n2a")
                    engine("n2a").tensor_tensor(n2a, c0s, c1s, OP.add)
                    S32.rel("c0s", "c1s")
                    n2 = S32.get("n2" + sfx)
                    engine("n2").tensor_tensor(n2, n2a, c2s, OP.add)
                    S32.rel("n2a", "c2s")
                return dict(ph1=ph1, tg=tg, n2="n2" + sfx, c0="c0" + sfx,
                            c1="c1" + sfx, c2="c2" + sfx, e0d="e0d" + sfx,
                            e1d="e1d" + sfx, u="u" + sfx, bx="b0" + sfx,
                            by="b1" + sfx, bz="b2" + sfx, sgn="sgn" + sfx,
                            rd16="rd16" + sfx, CBLK=CBLK, DIF=DIF)

            def emit_p2b(carry):
                ph1 = carry["ph1"]; tg = carry["tg"]
                mask = ph1["mask"]; u0d = ph1["u0d"]
                OUT = pio.tile([128, 3 * F], dt, tag="OUT", name="OUT")
                OUTv = OUT[:].rearrange("p (f c) -> p f c", c=3)
                wide = E.get("wide", 0)
                n2 = S32[carry["n2"]]
                if wide:
                    CBLK = carry["CBLK"]; DIF = carry["DIF"]
                    c0 = CBLK[:, 0:F]; c1 = CBLK[:, F:2 * F]; c2 = CBLK[:, 2 * F:3 * F]
                    e0d = DIF[:, 0:F]; e1d = DIF[:, F:2 * F]
                else:
                    c0 = S16[carry["c0"]]; c1 = S16[carry["c1"]]; c2 = S16[carry["c2"]]
                    e0d = S16[carry["e0d"]]; e1d = S16[carry["e1d"]]
                u = S16[carry["u"]]; sgn = S16[carry["sgn"]]; rd16 = S16[carry["rd16"]]
                bx = S16[carry["b0"] if "b0" in carry else carry["bx"]]
                by = S16[carry["by"]]; bz = S16[carry["bz"]]
                if E.get("tqfuse", 0):
                    tq2 = S32.get("tq2")
                    ve._custom_dve(TQ1, out=tq2, in0=n2, in1=c2,
                                   s0=_RC0, s1=_RC1)
                    S32.rel(carry["n2"])
                elif E.get("tq", "C") == "C":
                    tq2 = S32.get("tq2")
                    ve._custom_dve(DIV2S, out=tq2, in0=c2, in1=n2,
                                   s0=_RC0, s1=_RC1, imm2=2.0)
                    S32.rel(carry["n2"])
                else:
                    r2 = S32.get("r2")
                    ve.reciprocal_approx_fast(r2, n2)
                    S32.rel(carry["n2"])
                    tq2 = S32.get("tq2")
                    ve.scalar_tensor_tensor(tq2, c2, 2.0, r2, OP.mult, OP.mult)
                    S32.rel("r2")
                tq16 = S16.get("tq16")
                _tqscale = 2.0 if E.get("tqfuse", 0) else 1.0
                if E.get("tqconv", "V") == "A":
                    ae.activation(tq16, tq2, A.Copy, scale=_tqscale)
                else:
                    ve.tensor_scalar(tq16, tq2, _tqscale, None, op0=OP.mult)
                S32.rel("tq2")
                if wide:
                    DW = p16.tile([128, 3 * F], hp, tag=f"DW{tg % 2}",
                                  name=f"DW{tg % 2}")
                    tqb = tq16.unsqueeze(1).broadcast_to([128, 3, F])
                    DWv = DW[:].rearrange("p (c f) -> p c f", c=3)
                    CBv = CBLK[:].rearrange("p (c f) -> p c f", c=3)
                    ve.tensor_tensor(DWv, CBv, tqb, OP.mult)
                    S16.rel("tq16")
                    ve.tensor_scalar(DW[:, 2 * F:3 * F], DW[:, 2 * F:3 * F],
                                     -1.0, None, op0=OP.add)
                    e2 = DIF[:, 2 * F:3 * F]
                    ve.tensor_scalar(e2, u0d, 0.0, None, op0=OP.add)
                    mb = mask.unsqueeze(1).broadcast_to([128, 3, F])
                    DIFv = DIF[:].rearrange("p (c f) -> p c f", c=3)
                    ve.copy_predicated(DIFv, mb, DWv)
                    e0, e1 = e0d, e1d
                else:
                    d0 = S16.get("d0"); d1 = S16.get("d1"); t5 = S16.get("t5")
                    ve.tensor_tensor(d0, tq16, c0, OP.mult)
                    S16.rel(carry["c0"])
                    ve.tensor_tensor(d1, tq16, c1, OP.mult)
                    S16.rel(carry["c1"])
                    ve.tensor_tensor(t5, tq16, c2, OP.mult)
                    S16.rel(carry["c2"], "tq16")
                    d2p = S16.get("d2p")
                    ve.tensor_scalar(d2p, t5, -1.0, None, op0=OP.add)
                    S16.rel("t5")

                    if custom_fp16_out:
                        e2 = u0d
                    else:
                        e2 = S16.get("e2")
                        ve.tensor_scalar(e2, u0d, 0.0, None, op0=OP.add)
                    ve.copy_predicated(e0d, mask, d0)
                    S16.rel("d0")
                    ve.copy_predicated(e1d, mask, d1)
                    S16.rel("d1")
                    ve.copy_predicated(e2, mask, d2p)
                    S16.rel("d2p")
                    e0, e1 = e0d, e1d

                t6 = S16.get("t6"); t7 = S16.get("t7")
                engine("t6").tensor_tensor(t6, u, e0, OP.mult)
                engine("t7").tensor_tensor(t7, by, e1, OP.mult)
                H = S16.get("H")
                ve.tensor_tensor(H, t6, t7, OP.add)
                S16.rel("t6", "t7")
                G1 = S16.get("G1")
                ve.tensor_tensor(G1, H, rd16, OP.mult)
                S16.rel(carry["rd16"])
                G = S16.get("G")
                engine("gsub").tensor_tensor(G, G1, e2, OP.subtract)
                S16.rel("G1")
                gx = S16.get("gx")
                ve.tensor_tensor(gx, bx, G, OP.mult)
                engine("outx").tensor_tensor(OUTv[:, :, 0], e0, gx, OP.subtract)
                S16.rel("gx")
                gx2 = S16.get("gx2")
                engine("gx2").tensor_tensor(gx2, sgn, e1, OP.mult)
                S16.rel(carry["sgn"])
                Ht2 = S16.get("Ht2")
                engine("ht2").tensor_tensor(Ht2, by, G, OP.mult)
                S16.rel("G")
                engine("outy").tensor_tensor(OUTv[:, :, 1], gx2, Ht2, OP.subtract)
                S16.rel("gx2", "Ht2")
                gx3 = S16.get("gx3")
                engine("gx3").tensor_tensor(gx3, e2, bz, OP.mult)
                engine("outz").tensor_tensor(OUTv[:, :, 2], gx3, H, OP.subtract)
                S16.rel("gx3", "H", carry["u"])
                if not wide:
                    if not custom_fp16_out:
                        S16.rel("e2")
                    S16.rel(carry["e0d"], carry["e1d"])
                for c in ("bx", "by", "bz"):
                    S16.rel(carry[c])

                dqo = {"S": nc.sync, "A": nc.scalar, "G": nc.gpsimd,
                       "V": nc.vector}[E.get("dmaqo", "A")]
                dqo.dma_start(wo_r[tg], OUT[:])

            import contextlib
            from collections import deque
            depth = int(E.get("depth", 1))
            for rep in range(repeat):
                carries = deque()
                for tg in range(T):
                    hctx = tc.high_priority() if E.get("p1hi") else contextlib.nullcontext()
                    with hctx:
                        ph1 = emit_p1(tg)
                    if len(carries) >= depth:
                        emit_p2b(carries.popleft())
                    carries.append(emit_p2a(tg, ph1))
                while carries:
                    emit_p2b(carries.popleft())
                assert not S16.used, f"S16 leak: {list(S16.used)}"
                assert not S32.used, f"S32 leak: {list(S32.used)}" 

    nc.compile()
    return nc


# ---------------- host runner (self-contained deliverable) ----------------
NCORES = 8
_CACHE = {}


def _get_module(R):
    if R not in _CACHE:
        _CACHE[R] = build_module(
            R, F=512, num_devices=NCORES, nsc16=50, nsc32=12,
            eng=dict(carg="P", wadj="P", conv="A", rden="C", tq="C",
                     bcopy="A", tqconv="A", zrelu="A", n2a="V", n2="V",
                     gx2="P", tj="P", tqfuse=1, m2tc=1))
    return _CACHE[R]


def kernel(wi, n, albedo, metallic, alpha_x, alpha_y, sample):
    """Full-input MultiLobeSGGX sample(): shards rays across 8 NeuronCores,
    runs the Bass kernel, gathers the full [N,3] float32 output.
    alpha_y is unused (the module asserts alpha_x == alpha_y)."""
    from concourse.bass_utils import run_bass_kernel_spmd
    wi = np.ascontiguousarray(wi, dtype=np.float32)
    n = np.ascontiguousarray(n, dtype=np.float32)
    albedo = np.ascontiguousarray(albedo, dtype=np.float32)
    metallic = np.ascontiguousarray(metallic, dtype=np.float32)
    alpha_x = np.ascontiguousarray(alpha_x, dtype=np.float32)
    sample = np.ascontiguousarray(sample, dtype=np.float32)
    Nf = wi.shape[0]
    R = Nf // NCORES
    nc = _get_module(R)
    in_maps = []
    for c in range(NCORES):
        s = slice(c * R, (c + 1) * R)
        in_maps.append({
            "wi": wi[s], "n": n[s], "albedo": albedo[s],
            "metallic": metallic[s], "alpha_x": alpha_x[s], "sample": sample[s],
        })
    res = run_bass_kernel_spmd(nc, in_maps, core_ids=list(range(NCORES)))
    return np.concatenate([res.results[c]["wo"] for c in range(NCORES)], axis=0)



# revision 2
# speedup vs baseline: 1.1096x; 1.1096x over previous
"""MultiLobeSGGX.sample() Trainium2 Bass kernel v3.

Key structure vs v2 baseline:
- Planar I/O: host ships wi/n as fp16 [3,N], albedo/metallic f32 [3,N],
  alpha fp16 [N], sample split s0 f32 [N] / s1 fp16 [N]; output fp16 [3,N].
  All SBUF tiles are packed per component -> every fp16 DVE op runs 2x,
  no dtype-convert ops, DMA bytes drop 28%.
- Diffuse unification: with aeff=1 (beta=0) and u0 = (1-u0d)/2 the specular
  SGGX VNDF pipeline emits EXACTLY the uniform-hemisphere sample, so the
  whole diffuse branch + 3 output selects disappear; only u0/aeff/basis
  selects remain (copy_predicated on mask).
- Wide ops: same-ALU sites batched into one instruction over [128, 2F/3F]
  slices (dtab, bkj, uuvv, p12, D3, gxht) incl. stride-0 broadcasts.
- Pool engine only ever sees scalar_tensor_tensor/TensorScalar/is_gt forms
  (0.6 efficiency) instead of tensor_tensor add/mult (0.42).
- ACT: sins of a tile PAIR emitted back-to-back, sqrt-family of the pair
  afterwards -> 2 act-table loads per pair instead of 2 per tile; the
  3 sqrts feed from adjacent slices where batching helps (sq2 wide).
"""
import sys
sys.path.insert(0, '/opt/trn_rl_repo')
import numpy as np
import concourse.bass as bass
import concourse.bacc as bacc
import concourse.mybir as mybir
import concourse.tile as tile

dt = mybir.dt.float32
hp = mybir.dt.float16
u8 = mybir.dt.uint8
A = mybir.ActivationFunctionType
OP = mybir.AluOpType
PI = float(np.pi)
EPS = 1e-6

# ---------------- custom DVE ops (registered into dve_ops at import) -------
import concourse.dve_ops as dops
from concourse.dve_spec import (Spec, Src0, Src1, C0, C1, C2, Bin, AluOp,
                                minn, maxx, Zero, One, lower, _has_src1)
from concourse.dve_uop import DveOpSpec
from concourse.dve_table_gen import dve_ver_for

_RC0 = -0.23549792   # recip seed Chebyshev consts (see dve_ops.py)
_RC1 = 2.0017324


def _recip1(x):
    """seed + 1 Newton-Raphson pass on node x (5 pipeline stages)."""
    nx = Bin(AluOp.BITWISE_NOT, x, x)
    y0 = nx * C0
    return y0 * (C1 - x * y0)


def _np_recip1(x, c0=_RC0, c1=_RC1):
    x = np.asarray(x, np.float32)
    nx = (~x.view(np.int32)).view(np.float32)
    y0 = nx * np.float32(c0)
    return y0 * (np.float32(c1) - x * y0)


def _dve_max(a, b):
    r = np.maximum(a, b)
    r = np.where(np.isnan(a), b, r)
    return np.where(np.isnan(b), a, r)


def _dve_min(a, b):
    r = np.minimum(a, b)
    r = np.where(np.isnan(a), b, r)
    return np.where(np.isnan(b), a, r)


def _register(name, spec):
    if name in dops._SUB_OPCODE_FOR_NAME:
        return next(o for o in dops.OPS if o.name == name)
    row = dops._CUSTOM_DVE_ROW_BASE + len(dops.OPS)
    assert row < 0x20
    ver = dve_ver_for("TRN2")
    tmp = DveOpSpec(name=name, opcode=row, uops=lower(spec, ver=ver),
                    rd1_en=_has_src1(spec))
    op = dops.DveOp(name, spec, False, {ver: tmp.sha(ver)})
    dops.OPS.append(op)
    dops._SUB_OPCODE_FOR_NAME[name] = row
    dops.CUSTOM_DVE_SPECS[name] = spec
    return op


def _ref_divclip(in0, in1, s0, s1, imm2):
    y = _np_recip1(in1, s0, s1)
    p = in0.astype(np.float32) * y
    return _dve_min(_dve_max(p, np.float32(0.0)), np.float32(1.0))


# out = clip01(in0 * recip1(in1)); s0/s1 = recip seed consts
DIVCLIP = _register("ANT_DIVCLIP01", Spec(
    body=minn(maxx(Src0 * _recip1(Src1), Zero), One),
    reference=_ref_divclip))


def _ref_halfd(in0, in1, s0, s1, imm2):
    y = _np_recip1(np.asarray(in1, np.float32) * 2.0 + np.float32(imm2), s0, s1)
    return in0.astype(np.float32) * y


# out = in0 * recip1(2*in1 + imm2) = 0.5*in0/(in1 + imm2/2); imm2 = 2*eps.
# Diffuse u0 remap (1-u0d)/2; bounded by construction on kept lanes.
HALFD = _register("ANT_HALFD", Spec(
    body=Src0 * _recip1(Src1 + Src1 + C2),
    reference=_ref_halfd))


def _ref_lum2(in0, in1, s0, s1, imm2):
    return in0.astype(np.float32) * np.float32(s0) + in1.astype(np.float32) * np.float32(s1)


LUM2 = _register("ANT_LUM2", Spec(
    body=Src0 * C0 + Src1 * C1,
    reference=_ref_lum2))


def _ref_rden(in0, in1, s0, s1, imm2):
    x = np.asarray(in0, np.float32) + np.asarray(in1, np.float32)
    return _np_recip1(x, s0, s1)


RDEN = _register("ANT_RDEN", Spec(
    body=_recip1(Src0 + Src1),
    reference=_ref_rden))


def _ref_m2t(in0, in1, s0, s1, imm2):
    r = np.float32(1.0) - np.asarray(in0, np.float32) * np.asarray(in1, np.float32)
    return _dve_max(r, np.float32(0.0))


M2T = _register("ANT_M2T", Spec(
    body=maxx(One - Src0 * Src1, Zero),
    reference=_ref_m2t))


def _ref_sii(in0, in1, s0, s1, imm2):
    a2 = np.asarray(in1, np.float32)
    return a2 + (np.float32(1.0) - a2) * np.square(np.asarray(in0, np.float32))


SII = _register("ANT_SII", Spec(
    body=Src1 + (One - Src1) * (Src0 * Src0),
    reference=_ref_sii))


def _ref_sqsum2(in0, in1, s0, s1, imm2):
    a = np.asarray(in0, np.float32); b = np.asarray(in1, np.float32)
    return a * a + b * b


SQSUM2 = _register("ANT_SQSUM2", Spec(
    body=Src0 * Src0 + Src1 * Src1,
    reference=_ref_sqsum2))


def _ref_tq1(in0, in1, s0, s1, imm2):
    c2 = np.asarray(in1, np.float32)
    n2 = np.asarray(in0, np.float32) + c2 * c2
    return c2 * _np_recip1(n2, s0, s1)


# out = in1 * recip1(in0 + in1^2); the x2 reflect scale lands on ACT (Copy).
TQ1 = _register("ANT_TQ1", Spec(
    body=Src1 * _recip1(Src0 + Src1 * Src1),
    reference=_ref_tq1))


# ---------------------------------------------------------------------------
class Scratch:
    """Fixed slot pool with FIFO reuse (spreads WAR deps across slots)."""

    def __init__(self, pool, F, n, dtype, pfx, width=1):
        self.slots = [pool.tile([128, width * F], dtype, tag=f"{pfx}{i}",
                                name=f"{pfx}{i}")
                      for i in range(n)]
        self.free = list(range(n))
        self.used = {}

    def get(self, nm):
        i = self.free.pop(0)
        self.used[nm] = i
        return self.slots[i][:]

    def rel(self, *nms):
        for nm in nms:
            self.free.append(self.used.pop(nm))

    def __getitem__(self, nm):
        return self.slots[self.used[nm]][:]


DEFAULT_ENG = {}


def build_module(R, F=512, num_devices=8, io_bufs=2, eng=None, repeat=1):
    """R rays per core; F free-dim elements per tile (tile = 128*F rays)."""
    E = dict(DEFAULT_ENG)
    if eng:
        E.update(eng)
    assert R % (128 * F) == 0
    T = R // (128 * F)
    assert T % 2 == 0, "tile pairing needs even tile count"

    nc = bacc.Bacc("TRN2", target_bir_lowering=False, debug=False,
                   num_devices=num_devices)

    def register_const(val):
        t = nc.alloc_sbuf_tensor(f"uconst-{val}", [128, 1], dt)
        nc.gpsimd.memset(t.ap(), val)
        nc.const_aps.aps[(dt, float(val))] = t.ap()
    for v in (PI, PI / 2, 1.0, -1.0, EPS):
        register_const(v)
    nc.all_engine_barrier()

    wi = nc.dram_tensor("wi", [3, R], hp, kind="ExternalInput")
    nn = nc.dram_tensor("n", [3, R], hp, kind="ExternalInput")
    am = nc.dram_tensor("am", [6, R], dt, kind="ExternalInput")
    ax = nc.dram_tensor("alpha", [R], hp, kind="ExternalInput")
    s0t = nc.dram_tensor("s0", [R], dt, kind="ExternalInput")
    s1t = nc.dram_tensor("s1", [R], hp, kind="ExternalInput")
    wo = nc.dram_tensor("wo", [3, R], hp, kind="ExternalOutput")

    wi_r = wi[:, :].rearrange("c (t p f) -> t p c f", p=128, f=F)
    nn_r = nn[:, :].rearrange("c (t p f) -> t p c f", p=128, f=F)
    am_r = am[:, :].rearrange("c (t p f) -> t p c f", p=128, f=F)
    ax_r = ax[:].rearrange("(t p f) -> t p f", p=128, f=F)
    s0_r = s0t[:].rearrange("(t p f) -> t p f", p=128, f=F)
    s1_r = s1t[:].rearrange("(t p f) -> t p f", p=128, f=F)
    wo_r = wo[:, :].rearrange("c (t p f) -> t p c f", p=128, f=F)

    ve = nc.vector
    ge = nc.gpsimd
    ae = nc.scalar

    # Pool binary ops: plain tensor_tensor only -- scalar_tensor_tensor
    # (TensorScalarPtr w/ second tensor) fails the Pool ISA opcode check.
    def p_bin(out, a, b, op):
        ge.tensor_tensor(out, a, b, op)

    def v_or_p(site, default="V"):
        return E.get(site, default)

    def emit_bin(site, default, out, a, b, op):
        if v_or_p(site, default) == "V":
            ve.tensor_tensor(out, a, b, op)
        else:
            p_bin(out, a, b, op)

    grp = int(E.get("group", 4))
    assert T % grp == 0
    with tile.TileContext(nc) as tc:
        with (
            tc.tile_pool(name="ld", bufs=io_bufs) as pld,
            tc.tile_pool(name="cy", bufs=grp + int(E.get("cyx", 0))) as pcy,
            tc.tile_pool(name="ou", bufs=int(E.get("oub", 6))) as pou,
            tc.tile_pool(name="sc", bufs=1) as psc,
        ):
            S16 = Scratch(psc, F, int(E.get("n16", 30)), hp, "h")
            S32 = Scratch(psc, F, int(E.get("n32", 8)), dt, "f")
            W2 = Scratch(psc, F, int(E.get("nw2", 10)), hp, "w2", width=2)
            W3 = Scratch(psc, F, int(E.get("nw3", 9)), hp, "w3", width=3)
            F2 = Scratch(psc, F, int(E.get("nf2", 2)), dt, "g2", width=2)

            def emit_p1(tg):
                WI3 = pld.tile([128, 3 * F], hp, tag="WI3", name="WI3")[:]
                AM6 = pld.tile([128, 6 * F], dt, tag="AM6", name="AM6")[:]
                ALP = pld.tile([128, F], hp, tag="ALP", name="ALP")[:]
                S0 = pld.tile([128, F], dt, tag="S0", name="S0")[:]
                S1 = pld.tile([128, F], hp, tag="S1", name="S1")[:]
                N3 = pcy.tile([128, 3 * F], hp, tag="N3", name="N3")[:]
                B3 = pcy.tile([128, 3 * F], hp, tag="B3", name="B3")[:]
                MQ = pcy.tile([128, F], u8, tag="MQ", name="MQ")[:]
                CS2 = pcy.tile([128, 2 * F], hp, tag="CS2", name="CS2")[:]
                U0 = pcy.tile([128, F], hp, tag="U0", name="U0")[:]
                AEF = pcy.tile([128, F], hp, tag="AEF", name="AEF")[:]

                dq = nc.sync
                v3 = lambda t: t.rearrange("p (c f) -> p c f", c=3)
                # first-needed first: p1 computes on AM6/S0/S1 before WI3/N3
                dq.dma_start(AM6.rearrange("p (c f) -> p c f", c=6), am_r[tg])
                dq.dma_start(S0, s0_r[tg])
                dq.dma_start(S1, s1_r[tg])
                dq.dma_start(ALP, ax_r[tg])
                dq.dma_start(v3(WI3), wi_r[tg])
                dq.dma_start(v3(N3), nn_r[tg])
                dq.dma_start(v3(B3), nn_r[tg])   # b initialized to n by DMA

                WDS = F2.get("WDS")            # f32 pair (wd | ws)
                WD = WDS[:, 0:F]; WS = WDS[:, F:2 * F]
                TOTE = S32.get("TOTE"); TP = S32.get("TP")
                # views picking the (al_c, me_c) lane pairs of interleaved AM6
                amv = AM6.rearrange("p (g c f) -> p g c f", g=2, c=3)
                wv = WDS.rearrange("p (g f) -> p g f", g=2)
                ve._custom_dve(LUM2, out=wv, in0=amv[:, :, 0, :],
                               in1=amv[:, :, 1, :], s0=0.2126, s1=0.7152)
                ve.scalar_tensor_tensor(wv, amv[:, :, 2, :], 0.0722, wv,
                                        OP.mult, OP.add)
                # routing (f32, Pool by default)
                emit_bin("tote", "P", TOTE, WD, WS, OP.add)
                emit_bin("tp", "P", TP, S0, TOTE, OP.mult)
                # u8 mask must come from DVE: Pool integer TT needs matching
                # dtypes (f32 in / u8 out rejected by the BIR verifier).
                ve.tensor_tensor(MQ, WS, TP, OP.is_gt)
                NUM2 = S32.get("NUM2")
                emit_bin("num2", "P", NUM2, TOTE, TP, OP.subtract)
                S32.rel("TOTE")
                # u0: spec = clip01(tp/ws); diffuse = 0.5*(tote-tp)/(wd+eps)
                TMPS = S16.get("TMPS")
                ve._custom_dve(DIVCLIP, out=TMPS, in0=TP, in1=WS,
                               s0=_RC0, s1=_RC1)
                S32.rel("TP")
                ve._custom_dve(HALFD, out=U0, in0=NUM2, in1=WD,
                               s0=_RC0, s1=_RC1, imm2=2.0 * EPS)
                F2.rel("WDS")
                S32.rel("NUM2")
                ve.copy_predicated(U0, MQ, TMPS)
                S16.rel("TMPS")
                # aeff: diffuse lanes -> 1.0 (beta=0 makes the SGGX pipeline
                # reduce exactly to the uniform-hemisphere sample)
                ge.memset(AEF, 1.0)
                ve.copy_predicated(AEF, MQ, ALP)
                # basis select: B3 arrives as n via DMA; spec lanes get wi
                mb = MQ.unsqueeze(1).broadcast_to([128, 3, F])
                B3v = B3.rearrange("p (c f) -> p c f", c=3)
                WI3v = WI3.rearrange("p (c f) -> p c f", c=3)
                ve.copy_predicated(B3v, mb, WI3v)
                # phi: cos into CS2[0:F], sin into CS2[F:2F]
                WADJ = S16.get("WADJ"); CARG = S16.get("CARG")
                if v_or_p("wadj", "V") == "V":
                    ve.tensor_scalar(WADJ, S1, 0.75, None, op0=OP.is_gt)
                else:
                    ge.tensor_scalar(WADJ, S1, 0.75, None, op0=OP.is_gt)
                emit_bin("carg", "V", CARG, S1, WADJ, OP.subtract)
                S16.rel("WADJ")
                ae.activation(CS2[:, F:2 * F], S1, A.Sin, scale=-2.0 * PI, bias=PI)
                ae.activation(CS2[:, 0:F], CARG, A.Sin, scale=-2.0 * PI, bias=PI / 2)
                S16.rel("CARG")
                return dict(MQ=MQ, U0=U0, AEF=AEF, CS2=CS2, B3=B3, N3=N3)

            bcast2 = lambda t: t.unsqueeze(1).broadcast_to([128, 2, F])
            bcast3 = lambda t: t.unsqueeze(1).broadcast_to([128, 3, F])
            v2 = lambda t: t.rearrange("p (c f) -> p c f", c=2)

            def emit_p2(phl, tgs):
                """Op-level interleaved phase 2 across len(phl) tiles."""
                Q = len(phl)
                qr = range(Q)
                sx = [p[1] for p in phl]
                B3 = [p[0]["B3"] for p in phl]
                N3 = [p[0]["N3"] for p in phl]
                U0 = [p[0]["U0"] for p in phl]
                AEF = [p[0]["AEF"] for p in phl]
                CS2 = [p[0]["CS2"] for p in phl]
                bx = [t[:, 0:F] for t in B3]
                by = [t[:, F:2 * F] for t in B3]
                bz = [t[:, 2 * F:3 * F] for t in B3]
                nx = [t[:, 0:F] for t in N3]
                ny = [t[:, F:2 * F] for t in N3]
                nz = [t[:, 2 * F:3 * F] for t in N3]

                def G16(nm):
                    return [S16.get(nm + sx[q]) for q in qr]

                def R16(nm):
                    S16.rel(*[nm + s for s in sx])

                A2 = G16("A2"); BETA = G16("BETA"); OMU = G16("OMU")
                SGN = G16("SGN"); RD = G16("RD")
                for q in qr:
                    if E.get("a2", "A") == "V":
                        ve.tensor_tensor(A2[q], AEF[q], AEF[q], OP.mult)
                    else:
                        ae.activation(A2[q], AEF[q], A.Square)
                for q in qr:
                    if E.get("beta", "A") == "V":
                        ve.tensor_scalar(BETA[q], A2[q], -1.0, 1.0,
                                         op0=OP.mult, op1=OP.add)
                    else:
                        ae.activation(BETA[q], A2[q], A.Identity, scale=-1.0,
                                      bias=1.0)
                for q in qr:
                    if E.get("omu", "A") == "V":
                        ve.tensor_scalar(OMU[q], U0[q], -1.0, 1.0,
                                         op0=OP.mult, op1=OP.add)
                    else:
                        ae.activation(OMU[q], U0[q], A.Identity, scale=-1.0,
                                      bias=1.0)
                for q in qr:
                    if E.get("sgn", "A") == "V":
                        ve.tensor_scalar(SGN[q], bz[q], 0.0, 2.0,
                                         op0=OP.is_ge, op1=OP.mult)
                        ve.tensor_scalar(SGN[q], SGN[q], -1.0, None,
                                         op0=OP.add)
                    else:
                        ae.activation(SGN[q], bz[q], A.Sign, bias=EPS)
                for q in qr:
                    ve._custom_dve(RDEN, out=RD[q], in0=SGN[q], in1=bz[q],
                                   s0=_RC0, s1=_RC1)

                # frame dots
                DTAB = [W2.get("DTAB" + s) for s in sx]
                D2 = G16("D2"); TI = G16("TI")
                KJI3 = [W3.get("KJI3" + s) for s in sx]
                kn = [t[:, 0:F] for t in KJI3]
                jn = [t[:, F:2 * F] for t in KJI3]
                inn = [t[:, 2 * F:3 * F] for t in KJI3]
                for q in qr:
                    ve.tensor_tensor(DTAB[q], B3[q][:, 0:2 * F],
                                     N3[q][:, 0:2 * F], OP.mult)
                for q in qr:
                    emit_bin("d2", "V", D2[q], DTAB[q][:, 0:F],
                             DTAB[q][:, F:2 * F], OP.add)
                W2.rel(*["DTAB" + s for s in sx])
                for q in qr:
                    emit_bin("ti", "P", TI[q], bz[q], nz[q], OP.mult)
                for q in qr:
                    emit_bin("inn", "V", inn[q], D2[q], TI[q], OP.add)
                R16("TI")
                QD = G16("QD"); QQ = G16("QQ"); UU_ = G16("UU_")
                for q in qr:
                    emit_bin("qd", "V", QD[q], D2[q], RD[q], OP.mult)
                R16("D2")
                for q in qr:
                    emit_bin("Q", "V", QQ[q], QD[q], nz[q], OP.add)
                R16("QD")
                for q in qr:
                    emit_bin("u", "P", UU_[q], SGN[q], bx[q], OP.mult)
                TK = G16("TK"); TJ = G16("TJ"); TJ2 = G16("TJ2")
                for q in qr:
                    emit_bin("tk", "V", TK[q], UU_[q], QQ[q], OP.mult)
                for q in qr:
                    emit_bin("kn", "V", kn[q], nx[q], TK[q], OP.subtract)
                R16("TK")
                for q in qr:
                    emit_bin("tj", "P", TJ[q], SGN[q], ny[q], OP.mult)
                for q in qr:
                    emit_bin("tj2", "P", TJ2[q], by[q], QQ[q], OP.mult)
                R16("QQ")
                for q in qr:
                    emit_bin("jn", "V", jn[q], TJ[q], TJ2[q], OP.subtract)
                R16("TJ"); R16("TJ2")

                # S-matrix pieces
                BKJ2 = [W2.get("BKJ2" + s) for s in sx]
                bk = [t[:, 0:F] for t in BKJ2]
                bj = [t[:, F:2 * F] for t in BKJ2]
                for q in qr:
                    ve.tensor_tensor(v2(BKJ2[q]), bcast2(BETA[q]),
                                     v2(KJI3[q][:, 0:2 * F]), OP.mult)
                R16("BETA")
                M2 = G16("M2")
                if E.get("m2t", "A") == "A":
                    BKK = G16("BKK")
                    for q in qr:
                        emit_bin("bkk", "V", BKK[q], bk[q], kn[q], OP.mult)
                    for q in qr:
                        ae.activation(M2[q], BKK[q], A.Relu, scale=-1.0,
                                      bias=1.0)
                    R16("BKK")
                else:
                    for q in qr:
                        ve._custom_dve(M2T, out=M2[q], in0=bk[q], in1=kn[q])
                SQI2 = [W2.get("SQI2" + s) for s in sx]   # (tsq, Sii)
                tsq = [t[:, 0:F] for t in SQI2]
                Sii = [t[:, F:2 * F] for t in SQI2]
                for q in qr:
                    emit_bin("tsq", "V", tsq[q], U0[q], A2[q], OP.mult)
                for q in qr:
                    ve._custom_dve(SII, out=Sii[q], in0=inn[q], in1=A2[q])
                R16("A2")
                SQO2 = [W2.get("SQO2" + s) for s in sx]   # (squa, sqS)
                squa = [t[:, 0:F] for t in SQO2]
                sqS = [t[:, F:2 * F] for t in SQO2]
                for q in qr:
                    ae.activation(SQO2[q], SQI2[q], A.Sqrt)
                WMSQ = G16("WMSQ")
                for q in qr:
                    emit_bin("wmsq", "V", WMSQ[q], OMU[q], M2[q], OP.mult)
                R16("OMU")
                UVW3 = [W3.get("UVW3" + s) for s in sx]   # (uu, vv, wm_)
                uu = [t[:, 0:F] for t in UVW3]
                vv = [t[:, F:2 * F] for t in UVW3]
                wm_ = [t[:, 2 * F:3 * F] for t in UVW3]
                for q in qr:
                    ae.activation(wm_[q], WMSQ[q], A.Sqrt)
                R16("WMSQ")
                for q in qr:
                    ve.tensor_tensor(v2(UVW3[q][:, 0:2 * F]), bcast2(squa[q]),
                                     v2(CS2[q]), OP.mult)

                # c coefficients
                P2 = [W2.get("P2" + s) for s in sx]
                p1 = [t[:, 0:F] for t in P2]
                p2 = [t[:, F:2 * F] for t in P2]
                for q in qr:
                    ve.tensor_tensor(v2(P2[q]), v2(UVW3[q][:, F:3 * F]),
                                     v2(KJI3[q][:, F:3 * F]), OP.mult)
                PS = G16("PS"); TC0 = G16("TC0"); TKS = G16("TKS")
                C3 = [W3.get("C3" + s) for s in sx]
                c0 = [t[:, 0:F] for t in C3]
                c1 = [t[:, F:2 * F] for t in C3]
                c2 = [t[:, 2 * F:3 * F] for t in C3]
                for q in qr:
                    emit_bin("P_", "V", PS[q], p1[q], p2[q], OP.add)
                for q in qr:
                    emit_bin("tc0", "V", TC0[q], bk[q], PS[q], OP.mult)
                R16("PS")
                for q in qr:
                    emit_bin("tk_", "V", TKS[q], uu[q], sqS[q], OP.mult)
                W2.rel(*["SQO2" + s for s in sx])
                for q in qr:
                    emit_bin("c0", "V", c0[q], TC0[q], TKS[q], OP.add)
                R16("TC0"); R16("TKS")
                T3 = G16("T3"); T4 = G16("T4")
                for q in qr:
                    emit_bin("t3", "V", T3[q], vv[q], M2[q], OP.mult)
                R16("M2")
                for q in qr:
                    emit_bin("t4", "P", T4[q], p2[q], bj[q], OP.mult)
                W2.rel(*["P2" + s for s in sx])
                W2.rel(*["BKJ2" + s for s in sx])
                for q in qr:
                    emit_bin("c1", "V", c1[q], T3[q], T4[q], OP.add)
                R16("T3"); R16("T4")
                for q in qr:
                    emit_bin("c2", "V", c2[q], wm_[q], Sii[q], OP.mult)
                W2.rel(*["SQI2" + s for s in sx])
                W3.rel(*["KJI3" + s for s in sx])
                W3.rel(*["UVW3" + s for s in sx])

                # normalize + reflect coefficients
                N2 = [S32.get("N2" + s) for s in sx]
                TQF = [S32.get("TQF" + s) for s in sx]
                for q in qr:
                    ve._custom_dve(SQSUM2, out=N2[q], in0=c0[q], in1=c1[q])
                for q in qr:
                    ve._custom_dve(TQ1, out=TQF[q], in0=N2[q], in1=c2[q],
                                   s0=_RC0, s1=_RC1)
                S32.rel(*["N2" + s for s in sx])
                TQ = G16("TQ")
                for q in qr:
                    if E.get("tqc", "A") == "V":
                        ve.tensor_scalar(TQ[q], TQF[q], 2.0, None, op0=OP.mult)
                    else:
                        ae.activation(TQ[q], TQF[q], A.Copy, scale=2.0)
                S32.rel(*["TQF" + s for s in sx])
                D3 = [W3.get("D3" + s) for s in sx]
                d0 = [t[:, 0:F] for t in D3]
                d1 = [t[:, F:2 * F] for t in D3]
                d2p = [t[:, 2 * F:3 * F] for t in D3]
                for q in qr:
                    ve.tensor_tensor(D3[q].rearrange("p (c f) -> p c f", c=3),
                                     bcast3(TQ[q]),
                                     C3[q].rearrange("p (c f) -> p c f", c=3),
                                     OP.mult)
                R16("TQ")
                W3.rel(*["C3" + s for s in sx])
                for q in qr:
                    if E.get("d2p", "V") == "A":
                        ae.activation(d2p[q], d2p[q], A.Identity, bias=-1.0)
                    else:
                        ve.tensor_scalar(d2p[q], d2p[q], -1.0, None, op0=OP.add)

                # basis expansion; GH3 = (gx, ht2, H), D3 morphs into
                # (d0, gx2, gx3) via in-place writes, OUT3 = D3 - GH3 wide.
                T6 = G16("T6"); T7 = G16("T7")
                GH3 = [W3.get("GH3" + s) for s in sx]
                HH = [t[:, 2 * F:3 * F] for t in GH3]
                for q in qr:
                    emit_bin("t6", "V", T6[q], UU_[q], d0[q], OP.mult)
                R16("UU_")
                for q in qr:
                    emit_bin("t7", "P", T7[q], by[q], d1[q], OP.mult)
                for q in qr:
                    emit_bin("H", "V", HH[q], T6[q], T7[q], OP.add)
                R16("T6"); R16("T7")
                G1 = G16("G1"); GG = G16("GG")
                for q in qr:
                    emit_bin("G1", "V", G1[q], HH[q], RD[q], OP.mult)
                R16("RD")
                for q in qr:
                    emit_bin("G", "V", GG[q], G1[q], d2p[q], OP.subtract)
                R16("G1")
                for q in qr:
                    ve.tensor_tensor(v2(GH3[q][:, 0:2 * F]),
                                     v2(B3[q][:, 0:2 * F]),
                                     bcast2(GG[q]), OP.mult)
                R16("GG")
                O3 = [pou.tile([128, 3 * F], hp, tag="O3", name="O3")[:]
                      for q in qr]
                for q in qr:   # gx2 = sgn*d1 in place of d1
                    emit_bin("gx2", "P", d1[q], SGN[q], d1[q], OP.mult)
                R16("SGN")
                for q in qr:   # gx3 = d2p*bz in place of d2p
                    emit_bin("gx3", "V", d2p[q], d2p[q], bz[q], OP.mult)
                for q in qr:
                    if E.get("out3", "P") == "V":
                        ve.tensor_tensor(O3[q], D3[q], GH3[q], OP.subtract)
                    else:
                        p_bin(O3[q], D3[q], GH3[q], OP.subtract)
                W3.rel(*["GH3" + s for s in sx])
                W3.rel(*["D3" + s for s in sx])
                return [(tgs[q], O3[q]) for q in qr]

            def flush_outs(pend):
                dqo = {"S": nc.sync, "A": nc.scalar, "V": nc.vector,
                       "P": nc.gpsimd}[E.get("dmaqo", "S")]
                for tg_, O3_ in pend:
                    dqo.dma_start(wo_r[tg_],
                                  O3_.rearrange("p (c f) -> p c f", c=3))
                pend.clear()

            il = int(E.get("il", 2))
            assert grp % il == 0
            pend = []
            for rep in range(repeat):
                for tg in range(0, T, grp):
                    phs = []
                    for i in range(grp):
                        phs.append((emit_p1(tg + i), f"@{i}"))
                        if i == 0:
                            # out-DMAs of the previous group issue AFTER this
                            # group's first input DMAs: their data is long
                            # ready, so no queue-parking serialization.
                            flush_outs(pend)
                    for j in range(0, grp, il):
                        pend += emit_p2(phs[j:j + il],
                                        list(range(tg + j, tg + j + il)))
            flush_outs(pend)
            assert not S16.used and not S32.used, (S16.used, S32.used)
            assert not W2.used and not W3.used and not F2.used

    nc.compile()
    return nc


# ---------------- host runner (self-contained deliverable) ----------------
NCORES = 8
_CACHE = {}
SIM_KW = dict(F=512, io_bufs=2,
              eng=dict(d2p="A", group=2, il=2, oub=4, cyx=2,
                       ti="V", u="V", tj="V", tj2="V", out3="V",
                       gx2="P", tsq="P", wmsq="P", t4="P",
                       n16=28, nw2=9, nw3=9, n32=7))


def _get_module(R):
    if R not in _CACHE:
        _CACHE[R] = build_module(R, num_devices=NCORES, **SIM_KW)
    return _CACHE[R]


def kernel(wi, n, albedo, metallic, alpha_x, alpha_y, sample):
    """Full-input MultiLobeSGGX sample(): shards rays across 8 NeuronCores.
    alpha_y unused (module asserts alpha_x == alpha_y)."""
    from concourse.bass_utils import run_bass_kernel_spmd
    Nf = wi.shape[0]
    R = Nf // NCORES
    wi16 = np.ascontiguousarray(wi.T.astype(np.float16))
    n16 = np.ascontiguousarray(n.T.astype(np.float16))
    am = np.ascontiguousarray(
        np.concatenate([albedo.T, metallic.T], axis=0).astype(np.float32))
    ax16 = np.ascontiguousarray(alpha_x[:, 0].astype(np.float16))
    s0 = np.ascontiguousarray(sample[:, 0].astype(np.float32))
    s116 = np.ascontiguousarray(sample[:, 1].astype(np.float16))
    nc = _get_module(R)
    in_maps = []
    for c in range(NCORES):
        s = slice(c * R, (c + 1) * R)
        in_maps.append({
            "wi": np.ascontiguousarray(wi16[:, s]),
            "n": np.ascontiguousarray(n16[:, s]),
            "am": np.ascontiguousarray(am[:, s]),
            "alpha": ax16[s], "s0": s0[s], "s1": s116[s],
        })
    res = run_bass_kernel_spmd(nc, in_maps, core_ids=list(range(NCORES)))
    out = np.concatenate([res.results[c]["wo"] for c in range(NCORES)], axis=1)
    return np.ascontiguousarray(out.T.astype(np.float32))


# revision 3
# speedup vs baseline: 1.1369x; 1.0246x over previous
"""MultiLobeSGGX.sample() Trainium2 Bass kernel v3.

Key structure vs v2 baseline:
- Planar I/O: host ships wi/n as fp16 [3,N], albedo/metallic f32 [3,N],
  alpha fp16 [N], sample split s0 f32 [N] / s1 fp16 [N]; output fp16 [3,N].
  All SBUF tiles are packed per component -> every fp16 DVE op runs 2x,
  no dtype-convert ops, DMA bytes drop 28%.
- Diffuse unification: with aeff=1 (beta=0) and u0 = (1-u0d)/2 the specular
  SGGX VNDF pipeline emits EXACTLY the uniform-hemisphere sample, so the
  whole diffuse branch + 3 output selects disappear; only u0/aeff/basis
  selects remain (copy_predicated on mask).
- Wide ops: same-ALU sites batched into one instruction over [128, 2F/3F]
  slices (dtab, bkj, uuvv, p12, D3, gxht) incl. stride-0 broadcasts.
- Pool engine only ever sees scalar_tensor_tensor/TensorScalar/is_gt forms
  (0.6 efficiency) instead of tensor_tensor add/mult (0.42).
- ACT: sins of a tile PAIR emitted back-to-back, sqrt-family of the pair
  afterwards -> 2 act-table loads per pair instead of 2 per tile; the
  3 sqrts feed from adjacent slices where batching helps (sq2 wide).
"""
import sys
sys.path.insert(0, '/opt/trn_rl_repo')
import numpy as np
import concourse.bass as bass
import concourse.bacc as bacc
import concourse.mybir as mybir
import concourse.tile as tile

dt = mybir.dt.float32
hp = mybir.dt.float16
u8 = mybir.dt.uint8
A = mybir.ActivationFunctionType
OP = mybir.AluOpType
PI = float(np.pi)
EPS = 1e-6

# ---------------- custom DVE ops (registered into dve_ops at import) -------
import concourse.dve_ops as dops
from concourse.dve_spec import (Spec, Src0, Src1, C0, C1, C2, Bin, AluOp,
                                minn, maxx, Zero, One, lower, _has_src1)
from concourse.dve_uop import DveOpSpec
from concourse.dve_table_gen import dve_ver_for

_RC0 = -0.23549792   # recip seed Chebyshev consts (see dve_ops.py)
_RC1 = 2.0017324


def _recip1(x):
    """seed + 1 Newton-Raphson pass on node x (5 pipeline stages)."""
    nx = Bin(AluOp.BITWISE_NOT, x, x)
    y0 = nx * C0
    return y0 * (C1 - x * y0)


def _np_recip1(x, c0=_RC0, c1=_RC1):
    x = np.asarray(x, np.float32)
    nx = (~x.view(np.int32)).view(np.float32)
    y0 = nx * np.float32(c0)
    return y0 * (np.float32(c1) - x * y0)


def _dve_max(a, b):
    r = np.maximum(a, b)
    r = np.where(np.isnan(a), b, r)
    return np.where(np.isnan(b), a, r)


def _dve_min(a, b):
    r = np.minimum(a, b)
    r = np.where(np.isnan(a), b, r)
    return np.where(np.isnan(b), a, r)


def _register(name, spec):
    if name in dops._SUB_OPCODE_FOR_NAME:
        return next(o for o in dops.OPS if o.name == name)
    row = dops._CUSTOM_DVE_ROW_BASE + len(dops.OPS)
    assert row < 0x20
    ver = dve_ver_for("TRN2")
    tmp = DveOpSpec(name=name, opcode=row, uops=lower(spec, ver=ver),
                    rd1_en=_has_src1(spec))
    op = dops.DveOp(name, spec, False, {ver: tmp.sha(ver)})
    dops.OPS.append(op)
    dops._SUB_OPCODE_FOR_NAME[name] = row
    dops.CUSTOM_DVE_SPECS[name] = spec
    return op


def _ref_divclip(in0, in1, s0, s1, imm2):
    y = _np_recip1(in1, s0, s1)
    p = in0.astype(np.float32) * y
    return _dve_min(_dve_max(p, np.float32(0.0)), np.float32(1.0))


# out = clip01(in0 * recip1(in1)); s0/s1 = recip seed consts
DIVCLIP = _register("ANT_DIVCLIP01", Spec(
    body=minn(maxx(Src0 * _recip1(Src1), Zero), One),
    reference=_ref_divclip))


def _ref_halfd(in0, in1, s0, s1, imm2):
    y = _np_recip1(np.asarray(in1, np.float32) * 2.0 + np.float32(imm2), s0, s1)
    return in0.astype(np.float32) * y


# out = in0 * recip1(2*in1 + imm2) = 0.5*in0/(in1 + imm2/2); imm2 = 2*eps.
# Diffuse u0 remap (1-u0d)/2; bounded by construction on kept lanes.
HALFD = _register("ANT_HALFD", Spec(
    body=Src0 * _recip1(Src1 + Src1 + C2),
    reference=_ref_halfd))


def _ref_lum2(in0, in1, s0, s1, imm2):
    return in0.astype(np.float32) * np.float32(s0) + in1.astype(np.float32) * np.float32(s1)


LUM2 = _register("ANT_LUM2", Spec(
    body=Src0 * C0 + Src1 * C1,
    reference=_ref_lum2))


def _ref_rden(in0, in1, s0, s1, imm2):
    x = np.asarray(in0, np.float32) + np.asarray(in1, np.float32)
    return _np_recip1(x, s0, s1)


RDEN = _register("ANT_RDEN", Spec(
    body=_recip1(Src0 + Src1),
    reference=_ref_rden))


def _ref_m2t(in0, in1, s0, s1, imm2):
    r = np.float32(1.0) - np.asarray(in0, np.float32) * np.asarray(in1, np.float32)
    return _dve_max(r, np.float32(0.0))


M2T = _register("ANT_M2T", Spec(
    body=maxx(One - Src0 * Src1, Zero),
    reference=_ref_m2t))


def _ref_sii(in0, in1, s0, s1, imm2):
    a2 = np.asarray(in1, np.float32)
    return a2 + (np.float32(1.0) - a2) * np.square(np.asarray(in0, np.float32))


SII = _register("ANT_SII", Spec(
    body=Src1 + (One - Src1) * (Src0 * Src0),
    reference=_ref_sii))


def _ref_sqsum2(in0, in1, s0, s1, imm2):
    a = np.asarray(in0, np.float32); b = np.asarray(in1, np.float32)
    return a * a + b * b


SQSUM2 = _register("ANT_SQSUM2", Spec(
    body=Src0 * Src0 + Src1 * Src1,
    reference=_ref_sqsum2))


def _ref_tq1(in0, in1, s0, s1, imm2):
    c2 = np.asarray(in1, np.float32)
    n2 = np.asarray(in0, np.float32) + c2 * c2
    return c2 * _np_recip1(n2, s0, s1)


# out = in1 * recip1(in0 + in1^2); the x2 reflect scale lands on ACT (Copy).
TQ1 = _register("ANT_TQ1", Spec(
    body=Src1 * _recip1(Src0 + Src1 * Src1),
    reference=_ref_tq1))


# ---------------------------------------------------------------------------
class Scratch:
    """Fixed slot pool with FIFO reuse (spreads WAR deps across slots)."""

    def __init__(self, pool, F, n, dtype, pfx, width=1):
        self.slots = [pool.tile([128, width * F], dtype, tag=f"{pfx}{i}",
                                name=f"{pfx}{i}")
                      for i in range(n)]
        self.free = list(range(n))
        self.used = {}

    def get(self, nm):
        i = self.free.pop(0)
        self.used[nm] = i
        return self.slots[i][:]

    def rel(self, *nms):
        for nm in nms:
            self.free.append(self.used.pop(nm))

    def __getitem__(self, nm):
        return self.slots[self.used[nm]][:]


DEFAULT_ENG = {}


def build_module(R, F=512, num_devices=8, io_bufs=2, eng=None, repeat=1):
    """R rays per core; F free-dim elements per tile (tile = 128*F rays)."""
    E = dict(DEFAULT_ENG)
    if eng:
        E.update(eng)
    assert R % (128 * F) == 0
    T = R // (128 * F)
    assert T % 2 == 0, "tile pairing needs even tile count"

    nc = bacc.Bacc("TRN2", target_bir_lowering=False, debug=False,
                   num_devices=num_devices)

    def register_const(val):
        t = nc.alloc_sbuf_tensor(f"uconst-{val}", [128, 1], dt)
        nc.gpsimd.memset(t.ap(), val)
        nc.const_aps.aps[(dt, float(val))] = t.ap()
    for v in (PI, PI / 2, 1.0, -1.0, EPS):
        register_const(v)
    nc.all_engine_barrier()

    wi = nc.dram_tensor("wi", [3, R], hp, kind="ExternalInput")
    nn = nc.dram_tensor("n", [3, R], hp, kind="ExternalInput")
    am = nc.dram_tensor("am", [6, R], dt, kind="ExternalInput")
    ax = nc.dram_tensor("alpha", [R], hp, kind="ExternalInput")
    s0t = nc.dram_tensor("s0", [R], dt, kind="ExternalInput")
    s1t = nc.dram_tensor("s1", [R], hp, kind="ExternalInput")
    wo = nc.dram_tensor("wo", [3, R], hp, kind="ExternalOutput")

    wi_r = wi[:, :].rearrange("c (t p f) -> t p c f", p=128, f=F)
    nn_r = nn[:, :].rearrange("c (t p f) -> t p c f", p=128, f=F)
    am_r = am[:, :].rearrange("c (t p f) -> t p c f", p=128, f=F)
    ax_r = ax[:].rearrange("(t p f) -> t p f", p=128, f=F)
    s0_r = s0t[:].rearrange("(t p f) -> t p f", p=128, f=F)
    s1_r = s1t[:].rearrange("(t p f) -> t p f", p=128, f=F)
    wo_r = wo[:, :].rearrange("c (t p f) -> t p c f", p=128, f=F)

    ve = nc.vector
    ge = nc.gpsimd
    ae = nc.scalar

    # Pool binary ops: plain tensor_tensor only -- scalar_tensor_tensor
    # (TensorScalarPtr w/ second tensor) fails the Pool ISA opcode check.
    def p_bin(out, a, b, op):
        ge.tensor_tensor(out, a, b, op)

    def v_or_p(site, default="V"):
        return E.get(site, default)

    def emit_bin(site, default, out, a, b, op):
        if v_or_p(site, default) == "V":
            ve.tensor_tensor(out, a, b, op)
        else:
            p_bin(out, a, b, op)

    grp = int(E.get("group", 4))
    assert T % grp == 0
    with tile.TileContext(nc) as tc:
        with (
            tc.tile_pool(name="ld", bufs=io_bufs) as pld,
            tc.tile_pool(name="cy", bufs=grp + int(E.get("cyx", 0))) as pcy,
            tc.tile_pool(name="ou", bufs=int(E.get("oub", 6))) as pou,
            tc.tile_pool(name="sc", bufs=1) as psc,
        ):
            S16 = Scratch(psc, F, int(E.get("n16", 30)), hp, "h")
            S32 = Scratch(psc, F, int(E.get("n32", 8)), dt, "f")
            W2 = Scratch(psc, F, int(E.get("nw2", 10)), hp, "w2", width=2)
            W3 = Scratch(psc, F, int(E.get("nw3", 9)), hp, "w3", width=3)
            F2 = Scratch(psc, F, int(E.get("nf2", 2)), dt, "g2", width=2)

            def emit_p1(tg):
                WI3 = pld.tile([128, 3 * F], hp, tag="WI3", name="WI3")[:]
                AM6 = pld.tile([128, 6 * F], dt, tag="AM6", name="AM6")[:]
                ALP = pld.tile([128, F], hp, tag="ALP", name="ALP")[:]
                S0 = pld.tile([128, F], dt, tag="S0", name="S0")[:]
                S1 = pld.tile([128, F], hp, tag="S1", name="S1")[:]
                N3 = pcy.tile([128, 3 * F], hp, tag="N3", name="N3")[:]
                B3 = pcy.tile([128, 3 * F], hp, tag="B3", name="B3")[:]
                MQ = pcy.tile([128, F], u8, tag="MQ", name="MQ")[:]
                CS2 = pcy.tile([128, 2 * F], hp, tag="CS2", name="CS2")[:]
                U0 = pcy.tile([128, F], hp, tag="U0", name="U0")[:]
                AEF = pcy.tile([128, F], hp, tag="AEF", name="AEF")[:]

                dq = nc.sync
                v3 = lambda t: t.rearrange("p (c f) -> p c f", c=3)
                # first-needed first: p1 computes on AM6/S0/S1 before WI3/N3
                dq.dma_start(AM6.rearrange("p (c f) -> p c f", c=6), am_r[tg])
                dq.dma_start(S0, s0_r[tg])
                dq.dma_start(S1, s1_r[tg])
                dq.dma_start(ALP, ax_r[tg])
                dq.dma_start(v3(WI3), wi_r[tg])
                dq.dma_start(v3(N3), nn_r[tg])
                dq.dma_start(v3(B3), nn_r[tg])   # b initialized to n by DMA

                WDS = F2.get("WDS")            # f32 pair (wd | ws)
                WD = WDS[:, 0:F]; WS = WDS[:, F:2 * F]
                TOTE = S32.get("TOTE"); TP = S32.get("TP")
                # views picking the (al_c, me_c) lane pairs of interleaved AM6
                amv = AM6.rearrange("p (g c f) -> p g c f", g=2, c=3)
                wv = WDS.rearrange("p (g f) -> p g f", g=2)
                ve._custom_dve(LUM2, out=wv, in0=amv[:, :, 0, :],
                               in1=amv[:, :, 1, :], s0=0.2126, s1=0.7152)
                ve.scalar_tensor_tensor(wv, amv[:, :, 2, :], 0.0722, wv,
                                        OP.mult, OP.add)
                # routing (f32, Pool by default)
                emit_bin("tote", "P", TOTE, WD, WS, OP.add)
                emit_bin("tp", "P", TP, S0, TOTE, OP.mult)
                # u8 mask must come from DVE: Pool integer TT needs matching
                # dtypes (f32 in / u8 out rejected by the BIR verifier).
                ve.tensor_tensor(MQ, WS, TP, OP.is_gt)
                NUM2 = S32.get("NUM2")
                emit_bin("num2", "P", NUM2, TOTE, TP, OP.subtract)
                S32.rel("TOTE")
                # u0: spec = clip01(tp/ws); diffuse = 0.5*(tote-tp)/(wd+eps)
                TMPS = S16.get("TMPS")
                ve._custom_dve(DIVCLIP, out=TMPS, in0=TP, in1=WS,
                               s0=_RC0, s1=_RC1)
                S32.rel("TP")
                ve._custom_dve(HALFD, out=U0, in0=NUM2, in1=WD,
                               s0=_RC0, s1=_RC1, imm2=2.0 * EPS)
                F2.rel("WDS")
                S32.rel("NUM2")
                ve.copy_predicated(U0, MQ, TMPS)
                S16.rel("TMPS")
                # aeff: diffuse lanes -> 1.0 (beta=0 makes the SGGX pipeline
                # reduce exactly to the uniform-hemisphere sample)
                ge.memset(AEF, 1.0)
                ve.copy_predicated(AEF, MQ, ALP)
                # basis select: B3 arrives as n via DMA; spec lanes get wi
                mb = MQ.unsqueeze(1).broadcast_to([128, 3, F])
                B3v = B3.rearrange("p (c f) -> p c f", c=3)
                WI3v = WI3.rearrange("p (c f) -> p c f", c=3)
                ve.copy_predicated(B3v, mb, WI3v)
                # phi: cos into CS2[0:F], sin into CS2[F:2F]
                WADJ = S16.get("WADJ"); CARG = S16.get("CARG")
                if v_or_p("wadj", "V") == "V":
                    ve.tensor_scalar(WADJ, S1, 0.75, None, op0=OP.is_gt)
                else:
                    ge.tensor_scalar(WADJ, S1, 0.75, None, op0=OP.is_gt)
                emit_bin("carg", "V", CARG, S1, WADJ, OP.subtract)
                S16.rel("WADJ")
                ae.activation(CS2[:, F:2 * F], S1, A.Sin, scale=-2.0 * PI, bias=PI)
                ae.activation(CS2[:, 0:F], CARG, A.Sin, scale=-2.0 * PI, bias=PI / 2)
                S16.rel("CARG")
                return dict(MQ=MQ, U0=U0, AEF=AEF, CS2=CS2, B3=B3, N3=N3)

            bcast2 = lambda t: t.unsqueeze(1).broadcast_to([128, 2, F])
            bcast3 = lambda t: t.unsqueeze(1).broadcast_to([128, 3, F])
            v2 = lambda t: t.rearrange("p (c f) -> p c f", c=2)

            def emit_p2(phl, tgs):
                """Op-level interleaved phase 2 across len(phl) tiles."""
                Q = len(phl)
                qr = range(Q)
                sx = [p[1] for p in phl]
                B3 = [p[0]["B3"] for p in phl]
                N3 = [p[0]["N3"] for p in phl]
                U0 = [p[0]["U0"] for p in phl]
                AEF = [p[0]["AEF"] for p in phl]
                CS2 = [p[0]["CS2"] for p in phl]
                bx = [t[:, 0:F] for t in B3]
                by = [t[:, F:2 * F] for t in B3]
                bz = [t[:, 2 * F:3 * F] for t in B3]
                nx = [t[:, 0:F] for t in N3]
                ny = [t[:, F:2 * F] for t in N3]
                nz = [t[:, 2 * F:3 * F] for t in N3]

                def G16(nm):
                    return [S16.get(nm + sx[q]) for q in qr]

                def R16(nm):
                    S16.rel(*[nm + s for s in sx])

                A2 = G16("A2"); BETA = G16("BETA"); OMU = G16("OMU")
                SGN = G16("SGN"); RD = G16("RD")
                for q in qr:
                    if E.get("a2", "A") == "V":
                        ve.tensor_tensor(A2[q], AEF[q], AEF[q], OP.mult)
                    else:
                        ae.activation(A2[q], AEF[q], A.Square)
                for q in qr:
                    if E.get("beta", "A") == "V":
                        ve.tensor_scalar(BETA[q], A2[q], -1.0, 1.0,
                                         op0=OP.mult, op1=OP.add)
                    else:
                        ae.activation(BETA[q], A2[q], A.Identity, scale=-1.0,
                                      bias=1.0)
                for q in qr:
                    if E.get("omu", "A") == "V":
                        ve.tensor_scalar(OMU[q], U0[q], -1.0, 1.0,
                                         op0=OP.mult, op1=OP.add)
                    else:
                        ae.activation(OMU[q], U0[q], A.Identity, scale=-1.0,
                                      bias=1.0)
                for q in qr:
                    if E.get("sgn", "A") == "V":
                        ve.tensor_scalar(SGN[q], bz[q], 0.0, 2.0,
                                         op0=OP.is_ge, op1=OP.mult)
                        ve.tensor_scalar(SGN[q], SGN[q], -1.0, None,
                                         op0=OP.add)
                    else:
                        ae.activation(SGN[q], bz[q], A.Sign, bias=EPS)
                for q in qr:
                    ve._custom_dve(RDEN, out=RD[q], in0=SGN[q], in1=bz[q],
                                   s0=_RC0, s1=_RC1)

                # frame dots
                DTAB = [W2.get("DTAB" + s) for s in sx]
                D2 = G16("D2"); TI = G16("TI")
                KJI3 = [W3.get("KJI3" + s) for s in sx]
                kn = [t[:, 0:F] for t in KJI3]
                jn = [t[:, F:2 * F] for t in KJI3]
                inn = [t[:, 2 * F:3 * F] for t in KJI3]
                for q in qr:
                    ve.tensor_tensor(DTAB[q], B3[q][:, 0:2 * F],
                                     N3[q][:, 0:2 * F], OP.mult)
                for q in qr:
                    emit_bin("d2", "V", D2[q], DTAB[q][:, 0:F],
                             DTAB[q][:, F:2 * F], OP.add)
                W2.rel(*["DTAB" + s for s in sx])
                for q in qr:
                    emit_bin("ti", "P", TI[q], bz[q], nz[q], OP.mult)
                for q in qr:
                    emit_bin("inn", "V", inn[q], D2[q], TI[q], OP.add)
                R16("TI")
                QD = G16("QD"); QQ = G16("QQ"); UU_ = G16("UU_")
                for q in qr:
                    emit_bin("qd", "V", QD[q], D2[q], RD[q], OP.mult)
                R16("D2")
                for q in qr:
                    emit_bin("Q", "V", QQ[q], QD[q], nz[q], OP.add)
                R16("QD")
                for q in qr:
                    emit_bin("u", "P", UU_[q], SGN[q], bx[q], OP.mult)
                TK = G16("TK"); TJ = G16("TJ"); TJ2 = G16("TJ2")
                for q in qr:
                    emit_bin("tk", "V", TK[q], UU_[q], QQ[q], OP.mult)
                for q in qr:
                    emit_bin("kn", "V", kn[q], nx[q], TK[q], OP.subtract)
                R16("TK")
                for q in qr:
                    emit_bin("tj", "P", TJ[q], SGN[q], ny[q], OP.mult)
                for q in qr:
                    emit_bin("tj2", "P", TJ2[q], by[q], QQ[q], OP.mult)
                R16("QQ")
                for q in qr:
                    emit_bin("jn", "V", jn[q], TJ[q], TJ2[q], OP.subtract)
                R16("TJ"); R16("TJ2")

                # S-matrix pieces
                BKJ2 = [W2.get("BKJ2" + s) for s in sx]
                bk = [t[:, 0:F] for t in BKJ2]
                bj = [t[:, F:2 * F] for t in BKJ2]
                for q in qr:
                    ve.tensor_tensor(v2(BKJ2[q]), bcast2(BETA[q]),
                                     v2(KJI3[q][:, 0:2 * F]), OP.mult)
                R16("BETA")
                M2 = G16("M2")
                if E.get("m2t", "A") == "A":
                    BKK = G16("BKK")
                    for q in qr:
                        emit_bin("bkk", "V", BKK[q], bk[q], kn[q], OP.mult)
                    for q in qr:
                        ae.activation(M2[q], BKK[q], A.Relu, scale=-1.0,
                                      bias=1.0)
                    R16("BKK")
                else:
                    for q in qr:
                        ve._custom_dve(M2T, out=M2[q], in0=bk[q], in1=kn[q])
                SQI2 = [W2.get("SQI2" + s) for s in sx]   # (tsq, Sii)
                tsq = [t[:, 0:F] for t in SQI2]
                Sii = [t[:, F:2 * F] for t in SQI2]
                for q in qr:
                    emit_bin("tsq", "V", tsq[q], U0[q], A2[q], OP.mult)
                for q in qr:
                    ve._custom_dve(SII, out=Sii[q], in0=inn[q], in1=A2[q])
                R16("A2")
                SQO2 = [W2.get("SQO2" + s) for s in sx]   # (squa, sqS)
                squa = [t[:, 0:F] for t in SQO2]
                sqS = [t[:, F:2 * F] for t in SQO2]
                for q in qr:
                    ae.activation(SQO2[q], SQI2[q], A.Sqrt)
                WMSQ = G16("WMSQ")
                for q in qr:
                    emit_bin("wmsq", "V", WMSQ[q], OMU[q], M2[q], OP.mult)
                R16("OMU")
                UVW3 = [W3.get("UVW3" + s) for s in sx]   # (uu, vv, wm_)
                uu = [t[:, 0:F] for t in UVW3]
                vv = [t[:, F:2 * F] for t in UVW3]
                wm_ = [t[:, 2 * F:3 * F] for t in UVW3]
                for q in qr:
                    ae.activation(wm_[q], WMSQ[q], A.Sqrt)
                R16("WMSQ")
                for q in qr:
                    ve.tensor_tensor(v2(UVW3[q][:, 0:2 * F]), bcast2(squa[q]),
                                     v2(CS2[q]), OP.mult)

                # c coefficients
                P2 = [W2.get("P2" + s) for s in sx]
                p1 = [t[:, 0:F] for t in P2]
                p2 = [t[:, F:2 * F] for t in P2]
                for q in qr:
                    ve.tensor_tensor(v2(P2[q]), v2(UVW3[q][:, F:3 * F]),
                                     v2(KJI3[q][:, F:3 * F]), OP.mult)
                PS = G16("PS")
                C3 = [W3.get("C3" + s) for s in sx]
                c0 = [t[:, 0:F] for t in C3]
                c1 = [t[:, F:2 * F] for t in C3]
                c2 = [t[:, 2 * F:3 * F] for t in C3]
                # X2=(tc0|t3), Y2=(tk_|t4): c0c1 = X2+Y2 as one wide op
                X2 = [W2.get("X2" + s) for s in sx]
                Y2 = [W2.get("Y2" + s) for s in sx]
                TC0 = [t[:, 0:F] for t in X2]
                T3 = [t[:, F:2 * F] for t in X2]
                TKS = [t[:, 0:F] for t in Y2]
                T4 = [t[:, F:2 * F] for t in Y2]
                for q in qr:
                    emit_bin("P_", "V", PS[q], p1[q], p2[q], OP.add)
                for q in qr:
                    emit_bin("tc0", "V", TC0[q], bk[q], PS[q], OP.mult)
                R16("PS")
                for q in qr:
                    emit_bin("tk_", "V", TKS[q], uu[q], sqS[q], OP.mult)
                W2.rel(*["SQO2" + s for s in sx])
                for q in qr:
                    emit_bin("t3", "V", T3[q], vv[q], M2[q], OP.mult)
                R16("M2")
                for q in qr:
                    emit_bin("t4", "P", T4[q], p2[q], bj[q], OP.mult)
                W2.rel(*["P2" + s for s in sx])
                W2.rel(*["BKJ2" + s for s in sx])
                for q in qr:
                    ve.tensor_tensor(v2(C3[q][:, 0:2 * F]), v2(X2[q]),
                                     v2(Y2[q]), OP.add)
                W2.rel(*["X2" + s for s in sx])
                W2.rel(*["Y2" + s for s in sx])
                for q in qr:
                    emit_bin("c2", "V", c2[q], wm_[q], Sii[q], OP.mult)
                W2.rel(*["SQI2" + s for s in sx])
                W3.rel(*["KJI3" + s for s in sx])
                W3.rel(*["UVW3" + s for s in sx])

                # normalize + reflect coefficients
                N2 = [S32.get("N2" + s) for s in sx]
                TQF = [S32.get("TQF" + s) for s in sx]
                for q in qr:
                    ve._custom_dve(SQSUM2, out=N2[q], in0=c0[q], in1=c1[q])
                for q in qr:
                    ve._custom_dve(TQ1, out=TQF[q], in0=N2[q], in1=c2[q],
                                   s0=_RC0, s1=_RC1)
                S32.rel(*["N2" + s for s in sx])
                TQ = G16("TQ")
                for q in qr:
                    if E.get("tqc", "A") == "V":
                        ve.tensor_scalar(TQ[q], TQF[q], 2.0, None, op0=OP.mult)
                    else:
                        ae.activation(TQ[q], TQF[q], A.Copy, scale=2.0)
                S32.rel(*["TQF" + s for s in sx])
                D3 = [W3.get("D3" + s) for s in sx]
                d0 = [t[:, 0:F] for t in D3]
                d1 = [t[:, F:2 * F] for t in D3]
                d2p = [t[:, 2 * F:3 * F] for t in D3]
                for q in qr:
                    ve.tensor_tensor(D3[q].rearrange("p (c f) -> p c f", c=3),
                                     bcast3(TQ[q]),
                                     C3[q].rearrange("p (c f) -> p c f", c=3),
                                     OP.mult)
                R16("TQ")
                W3.rel(*["C3" + s for s in sx])
                for q in qr:
                    if E.get("d2p", "V") == "A":
                        ae.activation(d2p[q], d2p[q], A.Identity, bias=-1.0)
                    else:
                        ve.tensor_scalar(d2p[q], d2p[q], -1.0, None, op0=OP.add)

                # basis expansion; GH3 = (gx, ht2, H), D3 morphs into
                # (d0, gx2, gx3) via in-place writes, OUT3 = D3 - GH3 wide.
                T6 = G16("T6"); T7 = G16("T7")
                GH3 = [W3.get("GH3" + s) for s in sx]
                HH = [t[:, 2 * F:3 * F] for t in GH3]
                for q in qr:
                    emit_bin("t6", "V", T6[q], UU_[q], d0[q], OP.mult)
                R16("UU_")
                for q in qr:
                    emit_bin("t7", "P", T7[q], by[q], d1[q], OP.mult)
                for q in qr:
                    emit_bin("H", "V", HH[q], T6[q], T7[q], OP.add)
                R16("T6"); R16("T7")
                G1 = G16("G1"); GG = G16("GG")
                for q in qr:
                    emit_bin("G1", "V", G1[q], HH[q], RD[q], OP.mult)
                R16("RD")
                for q in qr:
                    emit_bin("G", "V", GG[q], G1[q], d2p[q], OP.subtract)
                R16("G1")
                for q in qr:
                    ve.tensor_tensor(v2(GH3[q][:, 0:2 * F]),
                                     v2(B3[q][:, 0:2 * F]),
                                     bcast2(GG[q]), OP.mult)
                R16("GG")
                O3 = [pou.tile([128, 3 * F], hp, tag="O3", name="O3")[:]
                      for q in qr]
                for q in qr:   # gx2 = sgn*d1 in place of d1
                    emit_bin("gx2", "P", d1[q], SGN[q], d1[q], OP.mult)
                R16("SGN")
                for q in qr:   # gx3 = d2p*bz in place of d2p
                    emit_bin("gx3", "V", d2p[q], d2p[q], bz[q], OP.mult)
                for q in qr:
                    if E.get("out3", "P") == "V":
                        ve.tensor_tensor(O3[q], D3[q], GH3[q], OP.subtract)
                    else:
                        p_bin(O3[q], D3[q], GH3[q], OP.subtract)
                W3.rel(*["GH3" + s for s in sx])
                W3.rel(*["D3" + s for s in sx])
                return [(tgs[q], O3[q]) for q in qr]

            def flush_outs(pend):
                dqo = {"S": nc.sync, "A": nc.scalar, "V": nc.vector,
                       "P": nc.gpsimd}[E.get("dmaqo", "S")]
                for tg_, O3_ in pend:
                    dqo.dma_start(wo_r[tg_],
                                  O3_.rearrange("p (c f) -> p c f", c=3))
                pend.clear()

            il = int(E.get("il", 2))
            assert grp % il == 0
            pend = []
            for rep in range(repeat):
                for tg in range(0, T, grp):
                    phs = []
                    for i in range(grp):
                        phs.append((emit_p1(tg + i), f"@{i}"))
                        if i == 0:
                            # out-DMAs of the previous group issue AFTER this
                            # group's first input DMAs: their data is long
                            # ready, so no queue-parking serialization.
                            flush_outs(pend)
                    for j in range(0, grp, il):
                        pend += emit_p2(phs[j:j + il],
                                        list(range(tg + j, tg + j + il)))
            flush_outs(pend)
            assert not S16.used and not S32.used, (S16.used, S32.used)
            assert not W2.used and not W3.used and not F2.used

    nc.compile()
    return nc


# ---------------- host runner (self-contained deliverable) ----------------
NCORES = 8
_CACHE = {}
SIM_KW = dict(F=512, io_bufs=2,
              eng=dict(d2p="A", group=2, il=2, oub=4, cyx=2,
                       ti="V", u="V", tj="V", tj2="V", t7="V", out3="V",
                       gx2="P", tsq="P", wmsq="P", t4="P",
                       n16=26, nw2=12, nw3=9, n32=7))


def _get_module(R):
    if R not in _CACHE:
        _CACHE[R] = build_module(R, num_devices=NCORES, **SIM_KW)
    return _CACHE[R]


def kernel(wi, n, albedo, metallic, alpha_x, alpha_y, sample):
    """Full-input MultiLobeSGGX sample(): shards rays across 8 NeuronCores.
    alpha_y unused (module asserts alpha_x == alpha_y)."""
    from concourse.bass_utils import run_bass_kernel_spmd
    Nf = wi.shape[0]
    R = Nf // NCORES
    wi16 = np.ascontiguousarray(wi.T.astype(np.float16))
    n16 = np.ascontiguousarray(n.T.astype(np.float16))
    am = np.ascontiguousarray(
        np.concatenate([albedo.T, metallic.T], axis=0).astype(np.float32))
    ax16 = np.ascontiguousarray(alpha_x[:, 0].astype(np.float16))
    s0 = np.ascontiguousarray(sample[:, 0].astype(np.float32))
    s116 = np.ascontiguousarray(sample[:, 1].astype(np.float16))
    nc = _get_module(R)
    in_maps = []
    for c in range(NCORES):
        s = slice(c * R, (c + 1) * R)
        in_maps.append({
            "wi": np.ascontiguousarray(wi16[:, s]),
            "n": np.ascontiguousarray(n16[:, s]),
            "am": np.ascontiguousarray(am[:, s]),
            "alpha": ax16[s], "s0": s0[s], "s1": s116[s],
        })
    res = run_bass_kernel_spmd(nc, in_maps, core_ids=list(range(NCORES)))
    out = np.concatenate([res.results[c]["wo"] for c in range(NCORES)], axis=1)
    return np.ascontiguousarray(out.T.astype(np.float32))


# revision 4
# speedup vs baseline: 1.1375x; 1.0005x over previous
"""MultiLobeSGGX.sample() Trainium2 Bass kernel v3.

Key structure vs v2 baseline:
- Planar I/O: host ships wi/n as fp16 [3,N], albedo/metallic f32 [3,N],
  alpha fp16 [N], sample split s0 f32 [N] / s1 fp16 [N]; output fp16 [3,N].
  All SBUF tiles are packed per component -> every fp16 DVE op runs 2x,
  no dtype-convert ops, DMA bytes drop 28%.
- Diffuse unification: with aeff=1 (beta=0) and u0 = (1-u0d)/2 the specular
  SGGX VNDF pipeline emits EXACTLY the uniform-hemisphere sample, so the
  whole diffuse branch + 3 output selects disappear; only u0/aeff/basis
  selects remain (copy_predicated on mask).
- Wide ops: same-ALU sites batched into one instruction over [128, 2F/3F]
  slices (dtab, bkj, uuvv, p12, D3, gxht) incl. stride-0 broadcasts.
- Pool engine only ever sees scalar_tensor_tensor/TensorScalar/is_gt forms
  (0.6 efficiency) instead of tensor_tensor add/mult (0.42).
- ACT: sins of a tile PAIR emitted back-to-back, sqrt-family of the pair
  afterwards -> 2 act-table loads per pair instead of 2 per tile; the
  3 sqrts feed from adjacent slices where batching helps (sq2 wide).
"""
import sys
sys.path.insert(0, '/opt/trn_rl_repo')
import numpy as np
import concourse.bass as bass
import concourse.bacc as bacc
import concourse.mybir as mybir
import concourse.tile as tile

dt = mybir.dt.float32
hp = mybir.dt.float16
u8 = mybir.dt.uint8
A = mybir.ActivationFunctionType
OP = mybir.AluOpType
PI = float(np.pi)
EPS = 1e-6

# ---------------- custom DVE ops (registered into dve_ops at import) -------
import concourse.dve_ops as dops
from concourse.dve_spec import (Spec, Src0, Src1, C0, C1, C2, Bin, AluOp,
                                minn, maxx, Zero, One, lower, _has_src1)
from concourse.dve_uop import DveOpSpec
from concourse.dve_table_gen import dve_ver_for

_RC0 = -0.23549792   # recip seed Chebyshev consts (see dve_ops.py)
_RC1 = 2.0017324


def _recip1(x):
    """seed + 1 Newton-Raphson pass on node x (5 pipeline stages)."""
    nx = Bin(AluOp.BITWISE_NOT, x, x)
    y0 = nx * C0
    return y0 * (C1 - x * y0)


def _np_recip1(x, c0=_RC0, c1=_RC1):
    x = np.asarray(x, np.float32)
    nx = (~x.view(np.int32)).view(np.float32)
    y0 = nx * np.float32(c0)
    return y0 * (np.float32(c1) - x * y0)


def _dve_max(a, b):
    r = np.maximum(a, b)
    r = np.where(np.isnan(a), b, r)
    return np.where(np.isnan(b), a, r)


def _dve_min(a, b):
    r = np.minimum(a, b)
    r = np.where(np.isnan(a), b, r)
    return np.where(np.isnan(b), a, r)


def _register(name, spec):
    if name in dops._SUB_OPCODE_FOR_NAME:
        return next(o for o in dops.OPS if o.name == name)
    row = dops._CUSTOM_DVE_ROW_BASE + len(dops.OPS)
    assert row < 0x20
    ver = dve_ver_for("TRN2")
    tmp = DveOpSpec(name=name, opcode=row, uops=lower(spec, ver=ver),
                    rd1_en=_has_src1(spec))
    op = dops.DveOp(name, spec, False, {ver: tmp.sha(ver)})
    dops.OPS.append(op)
    dops._SUB_OPCODE_FOR_NAME[name] = row
    dops.CUSTOM_DVE_SPECS[name] = spec
    return op


def _ref_divclip(in0, in1, s0, s1, imm2):
    y = _np_recip1(in1, s0, s1)
    p = in0.astype(np.float32) * y
    return _dve_min(_dve_max(p, np.float32(0.0)), np.float32(1.0))


# out = clip01(in0 * recip1(in1)); s0/s1 = recip seed consts
DIVCLIP = _register("ANT_DIVCLIP01", Spec(
    body=minn(maxx(Src0 * _recip1(Src1), Zero), One),
    reference=_ref_divclip))


def _ref_halfd(in0, in1, s0, s1, imm2):
    y = _np_recip1(np.asarray(in1, np.float32) * 2.0 + np.float32(imm2), s0, s1)
    return in0.astype(np.float32) * y


# out = in0 * recip1(2*in1 + imm2) = 0.5*in0/(in1 + imm2/2); imm2 = 2*eps.
# Diffuse u0 remap (1-u0d)/2; bounded by construction on kept lanes.
HALFD = _register("ANT_HALFD", Spec(
    body=Src0 * _recip1(Src1 + Src1 + C2),
    reference=_ref_halfd))


def _ref_lum2(in0, in1, s0, s1, imm2):
    return in0.astype(np.float32) * np.float32(s0) + in1.astype(np.float32) * np.float32(s1)


LUM2 = _register("ANT_LUM2", Spec(
    body=Src0 * C0 + Src1 * C1,
    reference=_ref_lum2))


def _ref_rden(in0, in1, s0, s1, imm2):
    x = np.asarray(in0, np.float32) + np.asarray(in1, np.float32)
    return _np_recip1(x, s0, s1)


RDEN = _register("ANT_RDEN", Spec(
    body=_recip1(Src0 + Src1),
    reference=_ref_rden))


def _ref_m2t(in0, in1, s0, s1, imm2):
    r = np.float32(1.0) - np.asarray(in0, np.float32) * np.asarray(in1, np.float32)
    return _dve_max(r, np.float32(0.0))


M2T = _register("ANT_M2T", Spec(
    body=maxx(One - Src0 * Src1, Zero),
    reference=_ref_m2t))


def _ref_sii(in0, in1, s0, s1, imm2):
    a2 = np.asarray(in1, np.float32)
    return a2 + (np.float32(1.0) - a2) * np.square(np.asarray(in0, np.float32))


SII = _register("ANT_SII", Spec(
    body=Src1 + (One - Src1) * (Src0 * Src0),
    reference=_ref_sii))


def _ref_sqsum2(in0, in1, s0, s1, imm2):
    a = np.asarray(in0, np.float32); b = np.asarray(in1, np.float32)
    return a * a + b * b


SQSUM2 = _register("ANT_SQSUM2", Spec(
    body=Src0 * Src0 + Src1 * Src1,
    reference=_ref_sqsum2))


def _ref_tq1(in0, in1, s0, s1, imm2):
    c2 = np.asarray(in1, np.float32)
    n2 = np.asarray(in0, np.float32) + c2 * c2
    return c2 * _np_recip1(n2, s0, s1)


# out = in1 * recip1(in0 + in1^2); the x2 reflect scale lands on ACT (Copy).
TQ1 = _register("ANT_TQ1", Spec(
    body=Src1 * _recip1(Src0 + Src1 * Src1),
    reference=_ref_tq1))


# ---------------------------------------------------------------------------
class Scratch:
    """Fixed slot pool with FIFO reuse (spreads WAR deps across slots)."""

    def __init__(self, pool, F, n, dtype, pfx, width=1):
        self.slots = [pool.tile([128, width * F], dtype, tag=f"{pfx}{i}",
                                name=f"{pfx}{i}")
                      for i in range(n)]
        self.free = list(range(n))
        self.used = {}

    def get(self, nm):
        i = self.free.pop(0)
        self.used[nm] = i
        return self.slots[i][:]

    def rel(self, *nms):
        for nm in nms:
            self.free.append(self.used.pop(nm))

    def __getitem__(self, nm):
        return self.slots[self.used[nm]][:]


DEFAULT_ENG = {}


def build_module(R, F=512, num_devices=8, io_bufs=2, eng=None, repeat=1):
    """R rays per core; F free-dim elements per tile (tile = 128*F rays)."""
    E = dict(DEFAULT_ENG)
    if eng:
        E.update(eng)
    assert R % (128 * F) == 0
    T = R // (128 * F)
    assert T % 2 == 0, "tile pairing needs even tile count"

    nc = bacc.Bacc("TRN2", target_bir_lowering=False, debug=False,
                   num_devices=num_devices)

    def register_const(val):
        t = nc.alloc_sbuf_tensor(f"uconst-{val}", [128, 1], dt)
        nc.gpsimd.memset(t.ap(), val)
        nc.const_aps.aps[(dt, float(val))] = t.ap()
    for v in (PI, PI / 2, 1.0, -1.0, EPS):
        register_const(v)
    nc.all_engine_barrier()

    wi = nc.dram_tensor("wi", [3, R], hp, kind="ExternalInput")
    nn = nc.dram_tensor("n", [3, R], hp, kind="ExternalInput")
    am = nc.dram_tensor("am", [6, R], dt, kind="ExternalInput")
    sa = nc.dram_tensor("salp", [2, R], hp, kind="ExternalInput")  # (s1|alpha)
    s0t = nc.dram_tensor("s0", [R], dt, kind="ExternalInput")
    wo = nc.dram_tensor("wo", [3, R], hp, kind="ExternalOutput")

    wi_r = wi[:, :].rearrange("c (t p f) -> t p c f", p=128, f=F)
    nn_r = nn[:, :].rearrange("c (t p f) -> t p c f", p=128, f=F)
    am_r = am[:, :].rearrange("c (t p f) -> t p c f", p=128, f=F)
    sa_r = sa[:, :].rearrange("c (t p f) -> t p c f", p=128, f=F)
    s0_r = s0t[:].rearrange("(t p f) -> t p f", p=128, f=F)
    wo_r = wo[:, :].rearrange("c (t p f) -> t p c f", p=128, f=F)
    wo_c_r = [wo[c, :].rearrange("(t p f) -> t p f", p=128, f=F)
              for c in range(3)]

    ve = nc.vector
    ge = nc.gpsimd
    ae = nc.scalar

    # Pool binary ops: plain tensor_tensor only -- scalar_tensor_tensor
    # (TensorScalarPtr w/ second tensor) fails the Pool ISA opcode check.
    def p_bin(out, a, b, op):
        ge.tensor_tensor(out, a, b, op)

    def v_or_p(site, default="V"):
        return E.get(site, default)

    def emit_bin(site, default, out, a, b, op):
        if v_or_p(site, default) == "V":
            ve.tensor_tensor(out, a, b, op)
        else:
            p_bin(out, a, b, op)

    grp = int(E.get("group", 4))
    assert T % grp == 0
    with tile.TileContext(nc) as tc:
        with (
            tc.tile_pool(name="ld", bufs=io_bufs) as pld,
            tc.tile_pool(name="cy", bufs=grp + int(E.get("cyx", 0))) as pcy,
            tc.tile_pool(name="ou", bufs=int(E.get("oub", 6))) as pou,
            tc.tile_pool(name="sc", bufs=1) as psc,
        ):
            S16 = Scratch(psc, F, int(E.get("n16", 30)), hp, "h")
            S32 = Scratch(psc, F, int(E.get("n32", 8)), dt, "f")
            W2 = Scratch(psc, F, int(E.get("nw2", 10)), hp, "w2", width=2)
            W3 = Scratch(psc, F, int(E.get("nw3", 9)), hp, "w3", width=3)
            F2 = Scratch(psc, F, int(E.get("nf2", 2)), dt, "g2", width=2)

            def emit_p1(tg):
                WI3 = pld.tile([128, 3 * F], hp, tag="WI3", name="WI3")[:]
                AM6 = pld.tile([128, 6 * F], dt, tag="AM6", name="AM6")[:]
                SAL = pld.tile([128, 2 * F], hp, tag="SAL", name="SAL")[:]
                S0 = pld.tile([128, F], dt, tag="S0", name="S0")[:]
                S1 = SAL[:, 0:F]; ALP = SAL[:, F:2 * F]
                N3 = pcy.tile([128, 3 * F], hp, tag="N3", name="N3")[:]
                B3 = pcy.tile([128, 3 * F], hp, tag="B3", name="B3")[:]
                MQ = pcy.tile([128, F], u8, tag="MQ", name="MQ")[:]
                CS2 = pcy.tile([128, 2 * F], hp, tag="CS2", name="CS2")[:]
                U0 = pcy.tile([128, F], hp, tag="U0", name="U0")[:]
                AEF = pcy.tile([128, F], hp, tag="AEF", name="AEF")[:]

                dq = nc.sync
                v3 = lambda t: t.rearrange("p (c f) -> p c f", c=3)
                first = (tg == 0)
                # first-needed first: p1 computes on AM6/S0/SAL before WI3/N3
                amv6 = AM6.rearrange("p (c f) -> p c f", c=6)
                if first:
                    # split the 12KB albedo|metallic transfer so the first
                    # luminance starts after the albedo half (~2.2us earlier)
                    dq.dma_start(amv6[:, 0:3, :], am_r[tg][:, 0:3, :])
                    dq.dma_start(S0, s0_r[tg])
                    dq.dma_start(amv6[:, 3:6, :], am_r[tg][:, 3:6, :])
                else:
                    dq.dma_start(amv6, am_r[tg])
                    dq.dma_start(S0, s0_r[tg])
                dq.dma_start(SAL.rearrange("p (c f) -> p c f", c=2), sa_r[tg])
                dq.dma_start(v3(WI3), wi_r[tg])
                dq.dma_start(v3(N3), nn_r[tg])
                # b initialized to n by the idle ACT engine (saves a DMA dup)
                ae.activation(B3, N3, A.Copy)

                WDS = F2.get("WDS")            # f32 pair (wd | ws)
                WD = WDS[:, 0:F]; WS = WDS[:, F:2 * F]
                TOTE = S32.get("TOTE"); TP = S32.get("TP")
                # views picking the (al_c, me_c) lane pairs of interleaved AM6
                amv = AM6.rearrange("p (g c f) -> p g c f", g=2, c=3)
                wv = WDS.rearrange("p (g f) -> p g f", g=2)
                if first:
                    ve._custom_dve(LUM2, out=WD, in0=AM6[:, 0:F],
                                   in1=AM6[:, F:2 * F], s0=0.2126, s1=0.7152)
                    ve.scalar_tensor_tensor(WD, AM6[:, 2 * F:3 * F], 0.0722,
                                            WD, OP.mult, OP.add)
                    ve._custom_dve(LUM2, out=WS, in0=AM6[:, 3 * F:4 * F],
                                   in1=AM6[:, 4 * F:5 * F],
                                   s0=0.2126, s1=0.7152)
                    ve.scalar_tensor_tensor(WS, AM6[:, 5 * F:6 * F], 0.0722,
                                            WS, OP.mult, OP.add)
                else:
                    ve._custom_dve(LUM2, out=wv, in0=amv[:, :, 0, :],
                                   in1=amv[:, :, 1, :], s0=0.2126, s1=0.7152)
                    ve.scalar_tensor_tensor(wv, amv[:, :, 2, :], 0.0722, wv,
                                            OP.mult, OP.add)
                # routing (f32, Pool by default)
                emit_bin("tote", "P", TOTE, WD, WS, OP.add)
                emit_bin("tp", "P", TP, S0, TOTE, OP.mult)
                # u8 mask must come from DVE: Pool integer TT needs matching
                # dtypes (f32 in / u8 out rejected by the BIR verifier).
                ve.tensor_tensor(MQ, WS, TP, OP.is_gt)
                NUM2 = S32.get("NUM2")
                emit_bin("num2", "P", NUM2, TOTE, TP, OP.subtract)
                S32.rel("TOTE")
                # u0: spec = clip01(tp/ws); diffuse = 0.5*(tote-tp)/(wd+eps)
                TMPS = S16.get("TMPS")
                ve._custom_dve(DIVCLIP, out=TMPS, in0=TP, in1=WS,
                               s0=_RC0, s1=_RC1)
                S32.rel("TP")
                ve._custom_dve(HALFD, out=U0, in0=NUM2, in1=WD,
                               s0=_RC0, s1=_RC1, imm2=2.0 * EPS)
                F2.rel("WDS")
                S32.rel("NUM2")
                ve.copy_predicated(U0, MQ, TMPS)
                S16.rel("TMPS")
                # aeff: diffuse lanes -> 1.0 (beta=0 makes the SGGX pipeline
                # reduce exactly to the uniform-hemisphere sample)
                ge.memset(AEF, 1.0)
                ve.copy_predicated(AEF, MQ, ALP)
                # basis select: B3 arrives as n via DMA; spec lanes get wi
                mb = MQ.unsqueeze(1).broadcast_to([128, 3, F])
                B3v = B3.rearrange("p (c f) -> p c f", c=3)
                WI3v = WI3.rearrange("p (c f) -> p c f", c=3)
                ve.copy_predicated(B3v, mb, WI3v)
                # phi: cos into CS2[0:F], sin into CS2[F:2F]
                WADJ = S16.get("WADJ"); CARG = S16.get("CARG")
                if v_or_p("wadj", "V") == "V":
                    ve.tensor_scalar(WADJ, S1, 0.75, None, op0=OP.is_gt)
                else:
                    ge.tensor_scalar(WADJ, S1, 0.75, None, op0=OP.is_gt)
                emit_bin("carg", "V", CARG, S1, WADJ, OP.subtract)
                S16.rel("WADJ")
                ae.activation(CS2[:, F:2 * F], S1, A.Sin, scale=-2.0 * PI, bias=PI)
                ae.activation(CS2[:, 0:F], CARG, A.Sin, scale=-2.0 * PI, bias=PI / 2)
                S16.rel("CARG")
                return dict(MQ=MQ, U0=U0, AEF=AEF, CS2=CS2, B3=B3, N3=N3)

            bcast2 = lambda t: t.unsqueeze(1).broadcast_to([128, 2, F])
            bcast3 = lambda t: t.unsqueeze(1).broadcast_to([128, 3, F])
            v2 = lambda t: t.rearrange("p (c f) -> p c f", c=2)

            def emit_p2(phl, tgs):
                """Op-level interleaved phase 2 across len(phl) tiles."""
                Q = len(phl)
                qr = range(Q)
                sx = [p[1] for p in phl]
                B3 = [p[0]["B3"] for p in phl]
                N3 = [p[0]["N3"] for p in phl]
                U0 = [p[0]["U0"] for p in phl]
                AEF = [p[0]["AEF"] for p in phl]
                CS2 = [p[0]["CS2"] for p in phl]
                bx = [t[:, 0:F] for t in B3]
                by = [t[:, F:2 * F] for t in B3]
                bz = [t[:, 2 * F:3 * F] for t in B3]
                nx = [t[:, 0:F] for t in N3]
                ny = [t[:, F:2 * F] for t in N3]
                nz = [t[:, 2 * F:3 * F] for t in N3]

                def G16(nm):
                    return [S16.get(nm + sx[q]) for q in qr]

                def R16(nm):
                    S16.rel(*[nm + s for s in sx])

                A2 = G16("A2"); BETA = G16("BETA"); OMU = G16("OMU")
                SGN = G16("SGN"); RD = G16("RD")
                for q in qr:
                    if E.get("a2", "A") == "V":
                        ve.tensor_tensor(A2[q], AEF[q], AEF[q], OP.mult)
                    else:
                        ae.activation(A2[q], AEF[q], A.Square)
                for q in qr:
                    if E.get("beta", "A") == "V":
                        ve.tensor_scalar(BETA[q], A2[q], -1.0, 1.0,
                                         op0=OP.mult, op1=OP.add)
                    else:
                        ae.activation(BETA[q], A2[q], A.Identity, scale=-1.0,
                                      bias=1.0)
                for q in qr:
                    if E.get("omu", "A") == "V":
                        ve.tensor_scalar(OMU[q], U0[q], -1.0, 1.0,
                                         op0=OP.mult, op1=OP.add)
                    else:
                        ae.activation(OMU[q], U0[q], A.Identity, scale=-1.0,
                                      bias=1.0)
                for q in qr:
                    if E.get("sgn", "A") == "V":
                        ve.tensor_scalar(SGN[q], bz[q], 0.0, 2.0,
                                         op0=OP.is_ge, op1=OP.mult)
                        ve.tensor_scalar(SGN[q], SGN[q], -1.0, None,
                                         op0=OP.add)
                    else:
                        ae.activation(SGN[q], bz[q], A.Sign, bias=EPS)
                for q in qr:
                    ve._custom_dve(RDEN, out=RD[q], in0=SGN[q], in1=bz[q],
                                   s0=_RC0, s1=_RC1)

                # frame dots
                DTAB = [W2.get("DTAB" + s) for s in sx]
                D2 = G16("D2"); TI = G16("TI")
                KJI3 = [W3.get("KJI3" + s) for s in sx]
                kn = [t[:, 0:F] for t in KJI3]
                jn = [t[:, F:2 * F] for t in KJI3]
                inn = [t[:, 2 * F:3 * F] for t in KJI3]
                for q in qr:
                    ve.tensor_tensor(DTAB[q], B3[q][:, 0:2 * F],
                                     N3[q][:, 0:2 * F], OP.mult)
                for q in qr:
                    emit_bin("d2", "V", D2[q], DTAB[q][:, 0:F],
                             DTAB[q][:, F:2 * F], OP.add)
                W2.rel(*["DTAB" + s for s in sx])
                for q in qr:
                    emit_bin("ti", "P", TI[q], bz[q], nz[q], OP.mult)
                for q in qr:
                    emit_bin("inn", "V", inn[q], D2[q], TI[q], OP.add)
                R16("TI")
                QD = G16("QD"); QQ = G16("QQ"); UU_ = G16("UU_")
                for q in qr:
                    emit_bin("qd", "V", QD[q], D2[q], RD[q], OP.mult)
                R16("D2")
                for q in qr:
                    emit_bin("Q", "V", QQ[q], QD[q], nz[q], OP.add)
                R16("QD")
                for q in qr:
                    emit_bin("u", "P", UU_[q], SGN[q], bx[q], OP.mult)
                TK = G16("TK"); TJ = G16("TJ"); TJ2 = G16("TJ2")
                for q in qr:
                    emit_bin("tk", "V", TK[q], UU_[q], QQ[q], OP.mult)
                for q in qr:
                    emit_bin("kn", "V", kn[q], nx[q], TK[q], OP.subtract)
                R16("TK")
                for q in qr:
                    emit_bin("tj", "P", TJ[q], SGN[q], ny[q], OP.mult)
                for q in qr:
                    emit_bin("tj2", "P", TJ2[q], by[q], QQ[q], OP.mult)
                R16("QQ")
                for q in qr:
                    emit_bin("jn", "V", jn[q], TJ[q], TJ2[q], OP.subtract)
                R16("TJ"); R16("TJ2")

                # S-matrix pieces
                BKJ2 = [W2.get("BKJ2" + s) for s in sx]
                bk = [t[:, 0:F] for t in BKJ2]
                bj = [t[:, F:2 * F] for t in BKJ2]
                for q in qr:
                    ve.tensor_tensor(v2(BKJ2[q]), bcast2(BETA[q]),
                                     v2(KJI3[q][:, 0:2 * F]), OP.mult)
                R16("BETA")
                M2 = G16("M2")
                if E.get("m2t", "A") == "A":
                    BKK = G16("BKK")
                    for q in qr:
                        emit_bin("bkk", "V", BKK[q], bk[q], kn[q], OP.mult)
                    for q in qr:
                        ae.activation(M2[q], BKK[q], A.Relu, scale=-1.0,
                                      bias=1.0)
                    R16("BKK")
                else:
                    for q in qr:
                        ve._custom_dve(M2T, out=M2[q], in0=bk[q], in1=kn[q])
                SQI2 = [W2.get("SQI2" + s) for s in sx]   # (tsq, Sii)
                tsq = [t[:, 0:F] for t in SQI2]
                Sii = [t[:, F:2 * F] for t in SQI2]
                for q in qr:
                    emit_bin("tsq", "V", tsq[q], U0[q], A2[q], OP.mult)
                for q in qr:
                    ve._custom_dve(SII, out=Sii[q], in0=inn[q], in1=A2[q])
                R16("A2")
                SQO2 = [W2.get("SQO2" + s) for s in sx]   # (squa, sqS)
                squa = [t[:, 0:F] for t in SQO2]
                sqS = [t[:, F:2 * F] for t in SQO2]
                for q in qr:
                    ae.activation(SQO2[q], SQI2[q], A.Sqrt)
                WMSQ = G16("WMSQ")
                for q in qr:
                    emit_bin("wmsq", "V", WMSQ[q], OMU[q], M2[q], OP.mult)
                R16("OMU")
                UVW3 = [W3.get("UVW3" + s) for s in sx]   # (uu, vv, wm_)
                uu = [t[:, 0:F] for t in UVW3]
                vv = [t[:, F:2 * F] for t in UVW3]
                wm_ = [t[:, 2 * F:3 * F] for t in UVW3]
                for q in qr:
                    ae.activation(wm_[q], WMSQ[q], A.Sqrt)
                R16("WMSQ")
                for q in qr:
                    ve.tensor_tensor(v2(UVW3[q][:, 0:2 * F]), bcast2(squa[q]),
                                     v2(CS2[q]), OP.mult)

                # c coefficients
                P2 = [W2.get("P2" + s) for s in sx]
                p1 = [t[:, 0:F] for t in P2]
                p2 = [t[:, F:2 * F] for t in P2]
                for q in qr:
                    ve.tensor_tensor(v2(P2[q]), v2(UVW3[q][:, F:3 * F]),
                                     v2(KJI3[q][:, F:3 * F]), OP.mult)
                PS = G16("PS")
                C3 = [W3.get("C3" + s) for s in sx]
                c0 = [t[:, 0:F] for t in C3]
                c1 = [t[:, F:2 * F] for t in C3]
                c2 = [t[:, 2 * F:3 * F] for t in C3]
                # X2=(tc0|t3), Y2=(tk_|t4): c0c1 = X2+Y2 as one wide op
                X2 = [W2.get("X2" + s) for s in sx]
                Y2 = [W2.get("Y2" + s) for s in sx]
                TC0 = [t[:, 0:F] for t in X2]
                T3 = [t[:, F:2 * F] for t in X2]
                TKS = [t[:, 0:F] for t in Y2]
                T4 = [t[:, F:2 * F] for t in Y2]
                for q in qr:
                    emit_bin("P_", "V", PS[q], p1[q], p2[q], OP.add)
                for q in qr:
                    emit_bin("tc0", "V", TC0[q], bk[q], PS[q], OP.mult)
                R16("PS")
                for q in qr:
                    emit_bin("tk_", "V", TKS[q], uu[q], sqS[q], OP.mult)
                W2.rel(*["SQO2" + s for s in sx])
                for q in qr:
                    emit_bin("t3", "V", T3[q], vv[q], M2[q], OP.mult)
                R16("M2")
                for q in qr:
                    emit_bin("t4", "P", T4[q], p2[q], bj[q], OP.mult)
                W2.rel(*["P2" + s for s in sx])
                W2.rel(*["BKJ2" + s for s in sx])
                for q in qr:
                    ve.tensor_tensor(v2(C3[q][:, 0:2 * F]), v2(X2[q]),
                                     v2(Y2[q]), OP.add)
                W2.rel(*["X2" + s for s in sx])
                W2.rel(*["Y2" + s for s in sx])
                for q in qr:
                    emit_bin("c2", "V", c2[q], wm_[q], Sii[q], OP.mult)
                W2.rel(*["SQI2" + s for s in sx])
                W3.rel(*["KJI3" + s for s in sx])
                W3.rel(*["UVW3" + s for s in sx])

                # normalize + reflect coefficients
                N2 = [S32.get("N2" + s) for s in sx]
                TQF = [S32.get("TQF" + s) for s in sx]
                for q in qr:
                    ve._custom_dve(SQSUM2, out=N2[q], in0=c0[q], in1=c1[q])
                for q in qr:
                    ve._custom_dve(TQ1, out=TQF[q], in0=N2[q], in1=c2[q],
                                   s0=_RC0, s1=_RC1)
                S32.rel(*["N2" + s for s in sx])
                TQ = G16("TQ")
                for q in qr:
                    if E.get("tqc", "A") == "V":
                        ve.tensor_scalar(TQ[q], TQF[q], 2.0, None, op0=OP.mult)
                    else:
                        ae.activation(TQ[q], TQF[q], A.Copy, scale=2.0)
                S32.rel(*["TQF" + s for s in sx])
                D3 = [W3.get("D3" + s) for s in sx]
                d0 = [t[:, 0:F] for t in D3]
                d1 = [t[:, F:2 * F] for t in D3]
                d2p = [t[:, 2 * F:3 * F] for t in D3]
                for q in qr:
                    ve.tensor_tensor(D3[q].rearrange("p (c f) -> p c f", c=3),
                                     bcast3(TQ[q]),
                                     C3[q].rearrange("p (c f) -> p c f", c=3),
                                     OP.mult)
                R16("TQ")
                W3.rel(*["C3" + s for s in sx])
                for q in qr:
                    if E.get("d2p", "V") == "A":
                        ae.activation(d2p[q], d2p[q], A.Identity, bias=-1.0)
                    else:
                        ve.tensor_scalar(d2p[q], d2p[q], -1.0, None, op0=OP.add)

                # basis expansion; GH3 = (gx, ht2, H), D3 morphs into
                # (d0, gx2, gx3) via in-place writes, OUT3 = D3 - GH3 wide.
                T6 = G16("T6"); T7 = G16("T7")
                GH3 = [W3.get("GH3" + s) for s in sx]
                HH = [t[:, 2 * F:3 * F] for t in GH3]
                for q in qr:
                    emit_bin("t6", "V", T6[q], UU_[q], d0[q], OP.mult)
                R16("UU_")
                for q in qr:
                    emit_bin("t7", "P", T7[q], by[q], d1[q], OP.mult)
                for q in qr:
                    emit_bin("H", "V", HH[q], T6[q], T7[q], OP.add)
                R16("T6"); R16("T7")
                G1 = G16("G1"); GG = G16("GG")
                for q in qr:
                    emit_bin("G1", "V", G1[q], HH[q], RD[q], OP.mult)
                R16("RD")
                for q in qr:
                    emit_bin("G", "V", GG[q], G1[q], d2p[q], OP.subtract)
                R16("G1")
                for q in qr:
                    ve.tensor_tensor(v2(GH3[q][:, 0:2 * F]),
                                     v2(B3[q][:, 0:2 * F]),
                                     bcast2(GG[q]), OP.mult)
                R16("GG")
                O3 = [pou.tile([128, 3 * F], hp, tag="O3", name="O3")[:]
                      for q in qr]
                for q in qr:   # gx2 = sgn*d1 in place of d1
                    emit_bin("gx2", "P", d1[q], SGN[q], d1[q], OP.mult)
                R16("SGN")
                for q in qr:   # gx3 = d2p*bz in place of d2p
                    emit_bin("gx3", "V", d2p[q], d2p[q], bz[q], OP.mult)
                if last:
                    # tail trim: narrow per-component outs, each DMA'd as
                    # soon as its component is ready (smaller final transfer)
                    dqo = {"S": nc.sync, "A": nc.scalar, "V": nc.vector,
                           "P": nc.gpsimd}[E.get("dmaqo", "S")]
                    for c in range(3):
                        for q in qr:
                            ve.tensor_tensor(O3[q][:, c * F:(c + 1) * F],
                                             D3[q][:, c * F:(c + 1) * F],
                                             GH3[q][:, c * F:(c + 1) * F],
                                             OP.subtract)
                        for q in qr:
                            dqo.dma_start(wo_c_r[c][tgs[q]],
                                          O3[q][:, c * F:(c + 1) * F])
                    W3.rel(*["GH3" + s for s in sx])
                    W3.rel(*["D3" + s for s in sx])
                    return []
                for q in qr:
                    if E.get("out3", "P") == "V":
                        ve.tensor_tensor(O3[q], D3[q], GH3[q], OP.subtract)
                    else:
                        p_bin(O3[q], D3[q], GH3[q], OP.subtract)
                W3.rel(*["GH3" + s for s in sx])
                W3.rel(*["D3" + s for s in sx])
                return [(tgs[q], O3[q]) for q in qr]

            def flush_outs(pend):
                dqo = {"S": nc.sync, "A": nc.scalar, "V": nc.vector,
                       "P": nc.gpsimd}[E.get("dmaqo", "S")]
                for tg_, O3_ in pend:
                    dqo.dma_start(wo_r[tg_],
                                  O3_.rearrange("p (c f) -> p c f", c=3))
                pend.clear()

            il = int(E.get("il", 2))
            assert grp % il == 0
            pend = []
            for rep in range(repeat):
                for tg in range(0, T, grp):
                    phs = []
                    for i in range(grp):
                        phs.append((emit_p1(tg + i), f"@{i}"))
                        if i == 0:
                            # out-DMAs of the previous group issue AFTER this
                            # group's first input DMAs: their data is long
                            # ready, so no queue-parking serialization.
                            flush_outs(pend)
                    for j in range(0, grp, il):
                        pend += emit_p2(phs[j:j + il],
                                        list(range(tg + j, tg + j + il)))
            flush_outs(pend)
            assert not S16.used and not S32.used, (S16.used, S32.used)
            assert not W2.used and not W3.used and not F2.used

    nc.compile()
    return nc


# ---------------- host runner (self-contained deliverable) ----------------
NCORES = 8
_CACHE = {}
SIM_KW = dict(F=512, io_bufs=2,
              eng=dict(d2p="A", group=2, il=2, oub=4, cyx=2,
                       ti="V", u="V", tj="V", tj2="V", t7="V", out3="V",
                       gx2="P", tsq="P", wmsq="P", t4="P",
                       n16=26, nw2=12, nw3=9, n32=7))


def _get_module(R):
    if R not in _CACHE:
        _CACHE[R] = build_module(R, num_devices=NCORES, **SIM_KW)
    return _CACHE[R]


def kernel(wi, n, albedo, metallic, alpha_x, alpha_y, sample):
    """Full-input MultiLobeSGGX sample(): shards rays across 8 NeuronCores.
    alpha_y unused (module asserts alpha_x == alpha_y)."""
    from concourse.bass_utils import run_bass_kernel_spmd
    Nf = wi.shape[0]
    R = Nf // NCORES
    wi16 = np.ascontiguousarray(wi.T.astype(np.float16))
    n16 = np.ascontiguousarray(n.T.astype(np.float16))
    am = np.ascontiguousarray(
        np.concatenate([albedo.T, metallic.T], axis=0).astype(np.float32))
    salp = np.ascontiguousarray(np.stack(
        [sample[:, 1].astype(np.float16), alpha_x[:, 0].astype(np.float16)]))
    s0 = np.ascontiguousarray(sample[:, 0].astype(np.float32))
    nc = _get_module(R)
    in_maps = []
    for c in range(NCORES):
        s = slice(c * R, (c + 1) * R)
        in_maps.append({
            "wi": np.ascontiguousarray(wi16[:, s]),
            "n": np.ascontiguousarray(n16[:, s]),
            "am": np.ascontiguousarray(am[:, s]),
            "salp": np.ascontiguousarray(salp[:, s]), "s0": s0[s],
        })
    res = run_bass_kernel_spmd(nc, in_maps, core_ids=list(range(NCORES)))
    out = np.concatenate([res.results[c]["wo"] for c in range(NCORES)], axis=1)
    return np.ascontiguousarray(out.T.astype(np.float32))
